# revision 57
# baseline (speedup 1.0000x reference)
"""Trainium2 Bass kernel for single-step decode attention with KV cache.

Problem: B=8, S=4 new tokens against a 4096-entry KV cache, H=32 heads,
HD=64, D=2048.  fp32 in/out.

Sharding: tensor-parallel over heads — each of the 8 cores owns 4 heads
(wq/wk/wv row-shards, wo col-shard, cache_k/cache_v head-shards) and
produces a partial [32, 2048] output; the host sums the 8 partials.

Structure (chunked streaming pipeline, all matmul data in bf16):
  * K/V cache, weights and x are cast to bf16 on the host — halves HBM
    traffic and keeps every matmul at 1 cycle/row.  Softmax statistics
    and PSUM accumulation stay fp32.
  * Softmax runs without max subtraction: for this problem the scaled
    scores are bounded (|s/8| <= ~6, exp <= ~400), so exp(s/8) is
    computed directly and a single 1/rowsum normalization is applied at
    the very end.  This removes the global max barrier entirely.
  * The cache is processed in 5 chunks (3x1024 + 2x512); each chunk is
    QK'd, exp'd, transposed, and immediately fed to the AV matmuls,
    which accumulate into four persistent PSUM banks (one per batch
    pair) across all chunks.  K, V DMA, QK, softmax, and AV therefore
    all overlap; the small final chunks shrink the serial tail.
  * QK packs 2 heads per matmul (2x64 = 128 contraction lanes) with
    zero-padded stationary operands; K arrives batch-pair-packed so
    each chunk DMA is one contiguous ~1 MB transfer.
  * AV packs 2 batches per matmul: V is host-interleaved by batch
    parity on the free axis, so one [128, 32] x [128, 512] matmul
    covers both batches (only the diagonal quadrants are kept).
  * The output projection and DMA are split in half so the first half
    streams out while the last batches are still in flight.
"""

import ml_dtypes
import numpy as np

import concourse.bass as bass
import concourse.mybir as mybir
import concourse.tile as tile
from concourse import bacc
from concourse.bass import ts
from concourse.masks import make_identity

F32 = mybir.dt.float32
BF16 = mybir.dt.bfloat16
NP_BF16 = ml_dtypes.bfloat16

B, S, D = 8, 4, 2048
H, HD = 32, 64
CACHE = 4096
NCORES = 8
HPC = H // NCORES            # heads per core = 4
PAIRS = HPC // 2             # head pairs per core = 2
NTOK = B * S                 # 32
DPC = HPC * HD               # 256 per-core model slice
KTOT = CACHE + S             # 4100
NBP = B // 2                 # batch pairs = 4

CHUNKS = [1024, 1024, 1024, 512, 512]
CUM = np.concatenate([[0], np.cumsum(CHUNKS)]).tolist()
# per-chunk exp/QK pieces: (global col, width, psum bank, rsp col)
PIECES = []
_pc = 0
for _ci, _cs in enumerate(CHUNKS):
    _lst, _o = [], 0
    while _o < _cs:
        _w = min(512, _cs - _o)
        _lst.append((CUM[_ci] + _o, _w, _pc % 2, _pc))
        _o += _w
        _pc += 1
    PIECES.append(_lst)
NEXP = _pc

_NC_CACHE = {}


def _build_nc():
    if "nc" in _NC_CACHE:
        return _NC_CACHE["nc"]

    nc = bacc.Bacc(None, target_bir_lowering=False)

    xT_d = nc.dram_tensor("xT", [128, 16, NTOK], BF16, kind="ExternalInput")
    wqkvT_d = nc.dram_tensor("wqkvT", [128, 3, 16, DPC], BF16, kind="ExternalInput")
    # K chunk-major and batch-pair packed: chunk c occupies flat columns
    # 4*CUM[c]..4*CUM[c+1], laid out as (j, p, k) per partition
    kT_d = nc.dram_tensor("kT", [NBP, 128, 4 * CACHE], BF16, kind="ExternalInput")
    # V interleaved by batch parity: [bpair, half, part, chunk, j(b%2), d]
    v_d = nc.dram_tensor("v", [NBP, 2, 128, 16, 2, DPC], BF16, kind="ExternalInput")
    mask8_d = nc.dram_tensor("mask8n", [128, S], F32, kind="ExternalInput")
    cosr_d = nc.dram_tensor("cosr", [NTOK, 128], F32, kind="ExternalInput")
    sinr_d = nc.dram_tensor("sinr", [NTOK, 128], F32, kind="ExternalInput")
    woT_d = nc.dram_tensor("woT", [DPC, D], BF16, kind="ExternalInput")
    out_d = nc.dram_tensor("out", [NTOK, D], F32, kind="ExternalOutput")

    EXP = mybir.ActivationFunctionType.Exp
    AX = mybir.AxisListType.X

    with tile.TileContext(nc) as tc:
        with (
            tc.tile_pool(name="const", bufs=1) as const,
            tc.tile_pool(name="wq_pool", bufs=6) as wq_pool,
            tc.tile_pool(name="kt_pool", bufs=6) as kt_pool,
            tc.tile_pool(name="v_pool", bufs=4) as v_pool,
            tc.tile_pool(name="attn_pool", bufs=1) as attn_pool,
        ):
            # ---- persistent SBUF tiles ----
            mask_sb = const.tile([128, S], F32, name="mask", tag="mask")
            cos_sb = const.tile([NTOK, 128], F32, name="cos", tag="cos")
            sin_sb = const.tile([NTOK, 128], F32, name="sin", tag="sin")
            id_sb = const.tile([128, 128], F32, name="ident", tag="ident")
            xT_sb = const.tile([128, 16, NTOK], BF16, name="xT", tag="xT")
            scores = const.tile([128, KTOT], F32, name="scores", tag="scores")
            probsT = const.tile([128, CACHE], BF16, name="probsT", tag="probsT")
            probsTn = const.tile([S, 128], BF16, name="probsTn", tag="probsTn")
            attnT_A = const.tile([128, NTOK], BF16, name="attnT_A", tag="attnT_A")
            attnT_B = const.tile([128, NTOK], BF16, name="attnT_B", tag="attnT_B")
            woT_sb = const.tile([128, 2, D], BF16, name="woT", tag="woT")
            xq_sb = const.tile([NTOK, DPC], F32, name="xq", tag="xq")
            xk_sb = const.tile([NTOK, DPC], F32, name="xk", tag="xk")
            xv32 = const.tile([NTOK, DPC], BF16, name="xv32", tag="xv32")
            xqT = [const.tile([128, NTOK], BF16, name=f"xqT{p}", tag=f"xqT{p}") for p in range(PAIRS)]
            xkT = [const.tile([128, NTOK], BF16, name=f"xkT{p}", tag=f"xkT{p}") for p in range(PAIRS)]
            zeros128 = const.tile([128, 128], F32, name="zeros128", tag="zeros128")
            lhsT = [
                [const.tile([128, 128], BF16, name=f"lhsT{b}_{p}", tag=f"lhsT{b}_{p}") for p in range(PAIRS)]
                for b in range(B)
            ]
            xvb = [
                const.tile([S, 2, DPC], BF16, name=f"xvb{bp}", tag=f"xvb{bp}")
                for bp in range(NBP)
            ]

            rsp = const.tile([128, NEXP + 1], F32, name="rsp", tag="rsp")
            rowsum = const.tile([128, 1], F32, name="rowsum", tag="rowsum")
            recip = const.tile([128, 1], F32, name="recip", tag="recip")
            # recip relocated to [(j, h, q), bpair] at partition base 0
            recip_f = const.tile([32, NBP], F32, name="recip_f", tag="recip_f")
            rope_t0 = const.tile([NTOK, 128], F32, name="rope_t0", tag="rope_t0")
            rope_t1 = const.tile([NTOK, 128], F32, name="rope_t1", tag="rope_t1")
            out_sb = [
                const.tile([16, D], F32, name=f"out{hb}", tag=f"out{hb}")
                for hb in range(2)
            ]

            # ---- phase A: constants + QKV projection + rope ----
            # sync queue order: wq -> wk -> wv (m-major: the q projection
            # and its rope/pack chain start as soon as wq lands) -> xT ->
            # kt stream
            nc.sync.dma_start(out=xT_sb, in_=xT_d[:])
            wts = [[None, None] for _ in range(3)]
            for m in range(3):
                for h in range(2):
                    wt = wq_pool.tile([128, 8, DPC], BF16, name="wt", tag="wt")
                    nc.sync.dma_start(out=wt, in_=wqkvT_d[:, m, 8 * h : 8 * h + 8, :])
                    wts[m][h] = wt
            nc.scalar.dma_start(out=cos_sb, in_=cosr_d[:])
            nc.scalar.dma_start(out=sin_sb, in_=sinr_d[:])
            nc.scalar.dma_start(out=mask_sb, in_=mask8_d[:])
            # prefetch all of V half-1 up front
            vt = [[None, None] for _ in range(NBP)]
            for bp in range(NBP):
                vt[bp][0] = v_pool.tile([128, 16, 2, DPC], BF16, name="vt", tag="vt")
                nc.scalar.dma_start(out=vt[bp][0], in_=v_d[bp, 0])
            nc.scalar.dma_start(
                out=woT_sb, in_=woT_d.rearrange("(c p) n -> p c n", p=128)
            )
            make_identity(nc, id_sb)

            psA_cm = tc.tile_pool(name="psA", bufs=1, space="PSUM")
            psA = psA_cm.__enter__()
            psT_cm = tc.tile_pool(name="psTA", bufs=2, space="PSUM")
            psT = psT_cm.__enter__()
            ps_q = psA.tile([NTOK, DPC], F32, name="ps_q", tag="ps_q")
            ps_k = psA.tile([NTOK, DPC], F32, name="ps_k", tag="ps_k")
            ps_v = psA.tile([NTOK, DPC], F32, name="ps_v", tag="ps_v")

            cos_r = cos_sb[:].rearrange("p (h i) -> p h i", h=HPC)
            sin_r = sin_sb[:].rearrange("p (h i) -> p h i", h=HPC)
            t0v = rope_t0[:].rearrange("p (h i) -> p h i", h=HPC)
            t1v = rope_t1[:].rearrange("p (h i) -> p h i", h=HPC)

            def proj(m, ps):
                for h in range(2):
                    for i in range(8):
                        c = 8 * h + i
                        nc.tensor.matmul(
                            ps, xT_sb[:, c, :], wts[m][h][:, i, :],
                            start=(c == 0), stop=(c == 15),
                        )

            def rope(ps, dst):
                # rope: projection columns are host-permuted to
                # (head, half, i) so the rotate pairs are contiguous
                # 32-wide blocks
                src = ps[:].rearrange("p (h t i) -> p h t i", h=HPC, t=2)
                dstv = dst[:].rearrange("p (h t i) -> p h t i", h=HPC, t=2)
                t0, t1 = src[:, :, 0, :], src[:, :, 1, :]
                nc.vector.tensor_mul(t0v, t0, cos_r)
                nc.vector.tensor_mul(t1v, t1, sin_r)
                nc.vector.tensor_sub(dstv[:, :, 0, :], t0v, t1v)
                nc.vector.tensor_mul(t0v, t0, sin_r)
                nc.vector.tensor_mul(t1v, t1, cos_r)
                nc.vector.tensor_add(dstv[:, :, 1, :], t0v, t1v)

            def transp(srct, dst):
                for p in range(PAIRS):
                    pt = psT.tile([128, NTOK], F32, name="ptA", tag="ptA")
                    nc.tensor.transpose(
                        pt, srct[:, ts(p, 128)], id_sb[0:NTOK, 0:NTOK]
                    )
                    nc.vector.tensor_copy(dst[p], pt)

            # zero-fill of the QK stationaries depends on nothing - do it
            # first so only the two data copies per (b, p) remain later
            nc.vector.memset(zeros128, 0.0)
            for b in range(B):
                for p in range(PAIRS):
                    nc.vector.tensor_copy(lhsT[b][p], zeros128)

            # q chain first: unblocks the cache QK stream
            proj(0, ps_q)
            rope(ps_q, xq_sb)
            transp(xq_sb, xqT)
            # zero-padded stationary QK operands: lhsT[b][p][dd, col] is
            # nonzero only for col = 16 b + 8 p + 4 h2 + q, h2 = dd // 64
            # (the stationary is zero-padded to all 128 output rows so every
            # (b, p) matmul accumulates into the same full-height PSUM bank)
            for b in range(B):
                for p in range(PAIRS):
                    t = lhsT[b][p]
                    base = 16 * b + 8 * p
                    nc.vector.tensor_copy(
                        t[0:64, base : base + S], xqT[p][0:64, ts(b, S)]
                    )
                    nc.vector.tensor_copy(
                        t[64:128, base + S : base + 8], xqT[p][64:128, ts(b, S)]
                    )
            # k chain
            proj(1, ps_k)
            rope(ps_k, xk_sb)
            transp(xk_sb, xkT)
            # v chain
            proj(2, ps_v)
            nc.vector.tensor_copy(xv32, ps_v)
            for bp in range(NBP):
                # per-bpair value rows relocated to partition base 0 (and
                # interleaved by batch parity on the free axis) so they can
                # be the rhs of the K=4 new-token AV matmul
                for jj in range(2):
                    b = 2 * bp + jj
                    nc.gpsimd.dma_start(
                        out=xvb[bp][:, jj, :], in_=xv32[S * b : S * (b + 1), :]
                    )

            # scores for the 4 new keys: raw scores + mask, exp'd right away
            # (no max subtraction anywhere — see module docstring)
            ps_n = psA.tile([128, S], F32, name="ps_n", tag="ps_n")
            for b in range(B):
                for p in range(PAIRS):
                    nc.tensor.matmul(
                        ps_n,
                        lhsT[b][p][:],
                        xkT[p][:, ts(b, S)],
                        start=(b == 0 and p == 0),
                        stop=(b == B - 1 and p == PAIRS - 1),
                    )
            nc.vector.tensor_add(scores[:, CACHE:KTOT], ps_n, mask_sb)
            nc.scalar.activation(
                scores[:, CACHE:KTOT], scores[:, CACHE:KTOT], EXP,
                scale=0.125, accum_out=rsp[:, NEXP : NEXP + 1],
            )
            ptn = psT.tile([S, 128], F32, name="ptN", tag="ptA")
            nc.tensor.transpose(ptn, scores[:, CACHE:KTOT], id_sb)
            nc.vector.tensor_copy(probsTn, ptn)

            psT_cm.__exit__(None, None, None)
            psA_cm.__exit__(None, None, None)

            # ---- chunked QK -> exp -> transpose -> AV pipeline ----
            psP_cm = tc.tile_pool(name="psP", bufs=1, space="PSUM")
            psP = psP_cm.__enter__()
            pe = [
                psP.tile([32, 2 * DPC], F32, name=f"pe{bp}", tag=f"pe{bp}")
                for bp in range(NBP)
            ]
            psD_cm = tc.tile_pool(name="psD", bufs=2, space="PSUM")
            psD = psD_cm.__enter__()
            psB_cm = tc.tile_pool(name="psB", bufs=1, space="PSUM")
            psB = psB_cm.__enter__()
            psb = [psB.tile([128, 512], F32, name=f"qk{kb}", tag=f"qk{kb}") for kb in range(2)]

            # the new-token AV term opens each accumulation (its probs
            # are ready in phase A), so the final chunk closes with no
            # extra tail matmul
            for bp in range(NBP):
                nc.tensor.matmul(
                    pe[bp],
                    probsTn[:, 32 * bp : 32 * bp + 32],
                    xvb[bp][:],
                    start=True,
                    stop=False,
                )

            def e_chunk(ci, bp, last):
                c0, c1 = CUM[ci], CUM[ci + 1]
                for g in range(c0 // 128, c1 // 128):
                    nc.tensor.matmul(
                        pe[bp],
                        probsT[:, 128 * g + 32 * bp : 128 * g + 32 * bp + 32],
                        vt[bp][g // 16][:, g % 16],
                        start=False,
                        stop=(last and g == c1 // 128 - 1),
                    )

            for ci, csize in enumerate(CHUNKS):
                c0, c1 = CUM[ci], CUM[ci + 1]
                # QK over this chunk: one packed (j, p, k) DMA per batch pair
                for bp in range(NBP):
                    kt = kt_pool.tile([128, 2, 2, 1024], BF16, name="kt", tag="kt")
                    ktv = kt[:, :, :, 0:csize]
                    nc.sync.dma_start(
                        out=ktv, in_=kT_d[bp, :, 4 * c0 : 4 * c1]
                    )
                    for jj in range(2):
                        b = 2 * bp + jj
                        for p in range(PAIRS):
                            for gcol, w, kb, _pi in PIECES[ci]:
                                o = gcol - c0
                                nc.tensor.matmul(
                                    psb[kb][:, 0:w],
                                    lhsT[b][p][:],
                                    ktv[:, jj, p, o : o + w],
                                    start=(bp == 0 and jj == 0 and p == 0),
                                    stop=(bp == NBP - 1 and jj == 1 and p == PAIRS - 1),
                                )
                # exp (scale folded into the activation, no max)
                for gcol, w, kb, pi in PIECES[ci]:
                    nc.scalar.activation(
                        scores[:, gcol : gcol + w], psb[kb][:, 0:w], EXP,
                        scale=0.125, accum_out=rsp[:, pi : pi + 1],
                    )
                if ci == len(CHUNKS) - 1:
                    # final normalization factors; the relocation DMAs run on
                    # gpsimd concurrently with the last AV matmuls
                    nc.vector.reduce_sum(rowsum, rsp[:], axis=AX)
                    nc.vector.reciprocal(recip, rowsum)
                    # relocations split across the (idle) sync queue and
                    # gpsimd so the chain is ~2x shorter
                    for b in range(B):
                        bp, jj = b // 2, b % 2
                        eng = nc.sync if jj == 0 else nc.gpsimd
                        eng.dma_start(
                            out=recip_f[16 * jj : 16 * jj + 16, bp : bp + 1],
                            in_=recip[16 * b : 16 * (b + 1), 0:1],
                        )
                # transpose probs chunk -> probsT (copies alternate between
                # DVE and Pool so neither engine becomes the bottleneck)
                for g in range(c0 // 128, c1 // 128):
                    pt = psD.tile([128, 128], F32, name="ptD", tag="ptD")
                    nc.tensor.transpose(pt, scores[:, ts(g, 128)], id_sb)
                    nc.vector.tensor_copy(probsT[:, ts(g, 128)], pt)
                # AV for this chunk (all but the last, which needs psE2/psF)
                if ci < len(CHUNKS) - 1:
                    for bp in range(NBP):
                        e_chunk(ci, bp, last=False)
                        if ci == 1:
                            # second V half arrives while chunk 2 QK streams
                            vt[bp][1] = v_pool.tile(
                                [128, 16, 2, DPC], BF16, name="vt", tag="vt"
                            )
                            nc.scalar.dma_start(out=vt[bp][1], in_=v_d[bp, 1])

            psB_cm.__exit__(None, None, None)
            psD_cm.__exit__(None, None, None)

            # ---- last chunk AV + normalize + attnT + split projection ----
            with (
                tc.tile_pool(name="psE2", bufs=2, space="PSUM") as psE2,
                tc.tile_pool(name="psF", bufs=2, space="PSUM") as psF,
            ):
                def proj_half(hb):
                    # output rows 16*hb .. 16*hb+16 (batches 4*hb..4*hb+4)
                    r0 = 16 * hb
                    for j in range(4):
                        po = psF.tile([16, 512], F32, name="po", tag="po")
                        nc.tensor.matmul(
                            po, attnT_A[:, r0 : r0 + 16], woT_sb[:, 0, ts(j, 512)],
                            start=True, stop=False,
                        )
                        nc.tensor.matmul(
                            po, attnT_B[:, r0 : r0 + 16], woT_sb[:, 1, ts(j, 512)],
                            start=False, stop=True,
                        )
                        nc.vector.tensor_copy(out_sb[hb][:, ts(j, 512)], po)
                    nc.sync.dma_start(
                        out=out_d[r0 : r0 + 16, :], in_=out_sb[hb][:]
                    )

                last_ci = len(CHUNKS) - 1
                # all AV matmuls + scales first, then all transposes: the
                # PE executes strictly in order, so interleaving would
                # bubble it on every DVE scale
                atss = []
                for bp in range(NBP):
                    e_chunk(last_ci, bp, last=True)
                    # engine APs must start at 32-partition boundaries, so
                    # each d-half is scaled as a full 32-row tile (16 rows
                    # are off-quadrant garbage, dropped by the copies below)
                    ats = []
                    for jj in range(2):
                        at = attn_pool.tile(
                            [32, DPC], F32, name="at", tag=f"at{bp}_{jj}"
                        )
                        rows = 16 if jj == 0 else 32
                        nc.vector.tensor_scalar_mul(
                            at[0:rows],
                            in0=pe[bp][0:rows, jj * DPC : (jj + 1) * DPC],
                            scalar1=recip_f[0:rows, bp : bp + 1],
                        )
                        ats.append(at)
                    atss.append(ats)
                for bp in range(NBP):
                    for g in range(2):
                        tgt = attnT_A if g == 0 else attnT_B
                        for jj in range(2):
                            pt32 = psE2.tile([128, 32], F32, name="pt32", tag="pt32")
                            nc.tensor.transpose(
                                pt32, atss[bp][jj][0:32, ts(g, 128)],
                                id_sb[0:32, 0:32],
                            )
                            base = 16 * jj + 8 * g
                            tok = S * (2 * bp + jj)
                            if g == 0:
                                nc.vector.tensor_copy(
                                    tgt[0:64, tok : tok + S],
                                    pt32[0:64, base : base + S],
                                )
                                nc.vector.tensor_copy(
                                    tgt[64:128, tok : tok + S],
                                    pt32[64:128, base + S : base + 8],
                                )
                            else:
                                nc.scalar.mul(
                                    tgt[0:64, tok : tok + S],
                                    pt32[0:64, base : base + S],
                                    1.0,
                                )
                                nc.scalar.mul(
                                    tgt[64:128, tok : tok + S],
                                    pt32[64:128, base + S : base + 8],
                                    1.0,
                                )
                    if bp == 1:
                        proj_half(0)
                    elif bp == 3:
                        proj_half(1)

            psP_cm.__exit__(None, None, None)

    nc.compile()
    _NC_CACHE["nc"] = nc
    return nc


def _rope_perm():
    # projection-output column permutation: (h, d=2i+half) -> (h, half, i)
    perm = np.empty(DPC, np.int64)
    for h in range(HPC):
        for half in range(2):
            for i in range(HD // 2):
                perm[h * HD + half * (HD // 2) + i] = h * HD + 2 * i + half
    return perm


def _prep_in_maps(inputs):
    x = np.ascontiguousarray(np.asarray(inputs["x"], np.float32))
    ck = np.asarray(inputs["cache_k"], np.float32)
    cv = np.asarray(inputs["cache_v"], np.float32)
    wq = np.asarray(inputs["wq"], np.float32)
    wk = np.asarray(inputs["wk"], np.float32)
    wv = np.asarray(inputs["wv"], np.float32)
    wo = np.asarray(inputs["wo"], np.float32)
    fc = np.asarray(inputs["freqs_cos"], np.float32)
    fs = np.asarray(inputs["freqs_sin"], np.float32)
    mask = np.asarray(inputs["mask"], np.float32)

    xT = np.ascontiguousarray(
        x.reshape(NTOK, D).T.reshape(16, 128, NTOK).transpose(1, 0, 2)
    ).astype(NP_BF16)
    cosr = np.ascontiguousarray(np.tile(fc, (B, HPC)))
    sinr = np.ascontiguousarray(np.tile(fs, (B, HPC)))
    mask8n = np.ascontiguousarray(np.tile(mask[0, 0][:, CACHE:] * 8.0, (NTOK, 1)))
    perm = _rope_perm()
    woT = wo.T

    in_maps = []
    for c in range(NCORES):
        hs = slice(HPC * c, HPC * (c + 1))
        ds = slice(DPC * c, DPC * (c + 1))
        wqT = wq[ds].T[:, perm]
        wkT = wk[ds].T[:, perm]
        wvT = wv[ds].T
        # m-major [p, m, c, n]: each m's weights are one contiguous
        # stream so the q chain starts before k/v arrive
        wqkvT = np.ascontiguousarray(
            np.stack(
                [
                    w.reshape(16, 128, DPC).transpose(1, 0, 2)
                    for w in (wqT, wkT, wvT)
                ],
                axis=1,
            )
        ).astype(NP_BF16)
        # [b, k, h, d] head-slice -> [b, pair, (h2, half, i), k]
        cks = ck[:, :, hs, :].reshape(B, CACHE, PAIRS, 2, HD // 2, 2)
        kTf = cks.transpose(0, 2, 3, 5, 4, 1).reshape(B, PAIRS, 128, CACHE)
        # chunk-major, batch-pair packed: [bp, part, (chunk | j, p, k)]
        kq = kTf.reshape(NBP, 2, PAIRS, 128, CACHE)
        blocks = [
            np.ascontiguousarray(
                kq[:, :, :, :, CUM[ci] : CUM[ci + 1]].transpose(0, 3, 1, 2, 4)
            ).reshape(NBP, 128, -1)
            for ci in range(len(CHUNKS))
        ]
        kT = np.ascontiguousarray(np.concatenate(blocks, axis=2)).astype(NP_BF16)
        # [b, hf, chunk, part, d] -> [bpair, hf, part, chunk, j, d]
        v = np.ascontiguousarray(
            cv[:, :, hs, :]
            .reshape(NBP, 2, 2, 16, 128, DPC)
            .transpose(0, 2, 4, 3, 1, 5)
        ).astype(NP_BF16)
        in_maps.append(
            dict(
                xT=xT,
                wqkvT=wqkvT,
                kT=kT,
                v=v,
                mask8n=mask8n,
                cosr=cosr,
                sinr=sinr,
                woT=np.ascontiguousarray(woT[ds]).astype(NP_BF16),
            )
        )
    return in_maps


def run_sharded(inputs, trace=False, **run_kwargs):
    """Build + run on 8 cores; returns (full_output, BassKernelResults)."""
    from concourse.bass_utils import run_bass_kernel_spmd

    nc = _build_nc()
    in_maps = _prep_in_maps(inputs)
    res = run_bass_kernel_spmd(
        nc, in_maps, core_ids=list(range(NCORES)), trace=trace, **run_kwargs
    )
    parts = np.stack([res.results[c]["out"] for c in range(NCORES)])
    out = parts.sum(axis=0, dtype=np.float32).reshape(B, S, D)
    return np.ascontiguousarray(out.astype(np.float32)), res


def kernel(**inputs):
    out, _ = run_sharded(inputs)
    return out


# revision 58
# speedup vs baseline: 1.0077x; 1.0077x over previous
"""Trainium2 Bass kernel for single-step decode attention with KV cache.

Problem: B=8, S=4 new tokens against a 4096-entry KV cache, H=32 heads,
HD=64, D=2048.  fp32 in/out.

Sharding: tensor-parallel over heads — each of the 8 cores owns 4 heads
(wq/wk/wv row-shards, wo col-shard, cache_k/cache_v head-shards) and
produces a partial [32, 2048] output; the host sums the 8 partials.

Structure (chunked streaming pipeline, all matmul data in bf16):
  * K/V cache, weights and x are cast to bf16 on the host — halves HBM
    traffic and keeps every matmul at 1 cycle/row.  Softmax statistics
    and PSUM accumulation stay fp32.
  * Softmax runs without max subtraction: for this problem the scaled
    scores are bounded (|s/8| <= ~6, exp <= ~400), so exp(s/8) is
    computed directly and a single 1/rowsum normalization is applied at
    the very end.  This removes the global max barrier entirely.
  * The cache is processed in 5 chunks (3x1024 + 2x512); each chunk is
    QK'd, exp'd, transposed, and immediately fed to the AV matmuls,
    which accumulate into four persistent PSUM banks (one per batch
    pair) across all chunks.  K, V DMA, QK, softmax, and AV therefore
    all overlap; the small final chunks shrink the serial tail.
  * QK packs 2 heads per matmul (2x64 = 128 contraction lanes) with
    zero-padded stationary operands; K arrives batch-pair-packed so
    each chunk DMA is one contiguous ~1 MB transfer.
  * AV packs 2 batches per matmul: V is host-interleaved by batch
    parity on the free axis, so one [128, 32] x [128, 512] matmul
    covers both batches (only the diagonal quadrants are kept).
  * The output projection and DMA are split in half so the first half
    streams out while the last batches are still in flight.
"""

import ml_dtypes
import numpy as np

import concourse.bass as bass
import concourse.mybir as mybir
import concourse.tile as tile
from concourse import bacc
from concourse.bass import ts
from concourse.masks import make_identity

F32 = mybir.dt.float32
BF16 = mybir.dt.bfloat16
NP_BF16 = ml_dtypes.bfloat16

B, S, D = 8, 4, 2048
H, HD = 32, 64
CACHE = 4096
NCORES = 8
HPC = H // NCORES            # heads per core = 4
PAIRS = HPC // 2             # head pairs per core = 2
NTOK = B * S                 # 32
DPC = HPC * HD               # 256 per-core model slice
KTOT = CACHE + S             # 4100
NBP = B // 2                 # batch pairs = 4

CHUNKS = [1024, 1024, 1024, 512, 512]
CUM = np.concatenate([[0], np.cumsum(CHUNKS)]).tolist()
# per-chunk exp/QK pieces: (global col, width, psum bank, rsp col)
PIECES = []
_pc = 0
for _ci, _cs in enumerate(CHUNKS):
    _lst, _o = [], 0
    while _o < _cs:
        _w = min(512, _cs - _o)
        _lst.append((CUM[_ci] + _o, _w, _pc % 2, _pc))
        _o += _w
        _pc += 1
    PIECES.append(_lst)
NEXP = _pc

_NC_CACHE = {}


def _build_nc():
    if "nc" in _NC_CACHE:
        return _NC_CACHE["nc"]

    nc = bacc.Bacc(None, target_bir_lowering=False)

    xT_d = nc.dram_tensor("xT", [128, 16, NTOK], BF16, kind="ExternalInput")
    wqkvT_d = nc.dram_tensor("wqkvT", [128, 3, 16, DPC], BF16, kind="ExternalInput")
    # K chunk-major and batch-pair packed: chunk c occupies flat columns
    # 4*CUM[c]..4*CUM[c+1], laid out as (j, p, k) per partition
    kT_d = nc.dram_tensor("kT", [NBP, 128, 4 * CACHE], BF16, kind="ExternalInput")
    # V interleaved by batch parity: [bpair, half, part, chunk, j(b%2), d]
    v_d = nc.dram_tensor("v", [NBP, 2, 128, 16, 2, DPC], BF16, kind="ExternalInput")
    mask8_d = nc.dram_tensor("mask8n", [128, S], F32, kind="ExternalInput")
    cosr_d = nc.dram_tensor("cosr", [NTOK, 128], F32, kind="ExternalInput")
    sinr_d = nc.dram_tensor("sinr", [NTOK, 128], F32, kind="ExternalInput")
    woT_d = nc.dram_tensor("woT", [DPC, D], BF16, kind="ExternalInput")
    out_d = nc.dram_tensor("out", [NTOK, D], F32, kind="ExternalOutput")

    EXP = mybir.ActivationFunctionType.Exp
    AX = mybir.AxisListType.X

    with tile.TileContext(nc) as tc:
        with (
            tc.tile_pool(name="const", bufs=1) as const,
            tc.tile_pool(name="wq_pool", bufs=6) as wq_pool,
            tc.tile_pool(name="kt_pool", bufs=6) as kt_pool,
            tc.tile_pool(name="v_pool", bufs=4) as v_pool,
            tc.tile_pool(name="attn_pool", bufs=1) as attn_pool,
        ):
            # ---- persistent SBUF tiles ----
            mask_sb = const.tile([128, S], F32, name="mask", tag="mask")
            cos_sb = const.tile([NTOK, 128], F32, name="cos", tag="cos")
            sin_sb = const.tile([NTOK, 128], F32, name="sin", tag="sin")
            id_sb = const.tile([128, 128], F32, name="ident", tag="ident")
            xT_sb = const.tile([128, 16, NTOK], BF16, name="xT", tag="xT")
            scores = const.tile([128, KTOT], F32, name="scores", tag="scores")
            probsT = const.tile([128, CACHE], BF16, name="probsT", tag="probsT")
            probsTn = const.tile([S, 128], BF16, name="probsTn", tag="probsTn")
            attnT_A = const.tile([128, NTOK], BF16, name="attnT_A", tag="attnT_A")
            attnT_B = const.tile([128, NTOK], BF16, name="attnT_B", tag="attnT_B")
            woT_sb = const.tile([128, 2, D], BF16, name="woT", tag="woT")
            xq_sb = const.tile([NTOK, DPC], F32, name="xq", tag="xq")
            xk_sb = const.tile([NTOK, DPC], F32, name="xk", tag="xk")
            xv32 = const.tile([NTOK, DPC], BF16, name="xv32", tag="xv32")
            xqT = [const.tile([128, NTOK], BF16, name=f"xqT{p}", tag=f"xqT{p}") for p in range(PAIRS)]
            xkT = [const.tile([128, NTOK], BF16, name=f"xkT{p}", tag=f"xkT{p}") for p in range(PAIRS)]
            zeros128 = const.tile([128, 128], F32, name="zeros128", tag="zeros128")
            lhsT = [
                [const.tile([128, 128], BF16, name=f"lhsT{b}_{p}", tag=f"lhsT{b}_{p}") for p in range(PAIRS)]
                for b in range(B)
            ]
            xvb = [
                const.tile([S, 2, DPC], BF16, name=f"xvb{bp}", tag=f"xvb{bp}")
                for bp in range(NBP)
            ]

            rsp = const.tile([128, NEXP + 1], F32, name="rsp", tag="rsp")
            rowsum = const.tile([128, 1], F32, name="rowsum", tag="rowsum")
            recip = const.tile([128, 1], F32, name="recip", tag="recip")
            # recip relocated to [(j, h, q), bpair] at partition base 0
            recip_f = const.tile([32, NBP], F32, name="recip_f", tag="recip_f")
            rope_t0 = const.tile([NTOK, 128], F32, name="rope_t0", tag="rope_t0")
            rope_t1 = const.tile([NTOK, 128], F32, name="rope_t1", tag="rope_t1")
            out_sb = [
                const.tile([16, D], F32, name=f"out{hb}", tag=f"out{hb}")
                for hb in range(2)
            ]

            # ---- phase A: constants + QKV projection + rope ----
            # sync queue order: wq -> wk -> wv (m-major: the q projection
            # and its rope/pack chain start as soon as wq lands) -> xT ->
            # kt stream
            nc.sync.dma_start(out=xT_sb, in_=xT_d[:])
            wts = [[None, None] for _ in range(3)]
            for m in range(3):
                for h in range(2):
                    wt = wq_pool.tile([128, 8, DPC], BF16, name="wt", tag="wt")
                    nc.sync.dma_start(out=wt, in_=wqkvT_d[:, m, 8 * h : 8 * h + 8, :])
                    wts[m][h] = wt
            nc.scalar.dma_start(out=cos_sb, in_=cosr_d[:])
            nc.scalar.dma_start(out=sin_sb, in_=sinr_d[:])
            nc.scalar.dma_start(out=mask_sb, in_=mask8_d[:])
            # prefetch all of V half-1 up front
            vt = [[None, None] for _ in range(NBP)]
            for bp in range(NBP):
                vt[bp][0] = v_pool.tile([128, 16, 2, DPC], BF16, name="vt", tag="vt")
                nc.scalar.dma_start(out=vt[bp][0], in_=v_d[bp, 0])
            nc.scalar.dma_start(
                out=woT_sb, in_=woT_d.rearrange("(c p) n -> p c n", p=128)
            )
            make_identity(nc, id_sb)

            psA_cm = tc.tile_pool(name="psA", bufs=1, space="PSUM")
            psA = psA_cm.__enter__()
            psT_cm = tc.tile_pool(name="psTA", bufs=2, space="PSUM")
            psT = psT_cm.__enter__()
            ps_q = psA.tile([NTOK, DPC], F32, name="ps_q", tag="ps_q")
            ps_k = psA.tile([NTOK, DPC], F32, name="ps_k", tag="ps_k")
            ps_v = psA.tile([NTOK, DPC], F32, name="ps_v", tag="ps_v")

            cos_r = cos_sb[:].rearrange("p (h i) -> p h i", h=HPC)
            sin_r = sin_sb[:].rearrange("p (h i) -> p h i", h=HPC)
            t0v = rope_t0[:].rearrange("p (h i) -> p h i", h=HPC)
            t1v = rope_t1[:].rearrange("p (h i) -> p h i", h=HPC)

            def proj(m, ps):
                for h in range(2):
                    for i in range(8):
                        c = 8 * h + i
                        nc.tensor.matmul(
                            ps, xT_sb[:, c, :], wts[m][h][:, i, :],
                            start=(c == 0), stop=(c == 15),
                        )

            def rope(ps, dst):
                # rope: projection columns are host-permuted to
                # (head, half, i) so the rotate pairs are contiguous
                # 32-wide blocks
                src = ps[:].rearrange("p (h t i) -> p h t i", h=HPC, t=2)
                dstv = dst[:].rearrange("p (h t i) -> p h t i", h=HPC, t=2)
                t0, t1 = src[:, :, 0, :], src[:, :, 1, :]
                nc.vector.tensor_mul(t0v, t0, cos_r)
                nc.vector.tensor_mul(t1v, t1, sin_r)
                nc.vector.tensor_sub(dstv[:, :, 0, :], t0v, t1v)
                nc.vector.tensor_mul(t0v, t0, sin_r)
                nc.vector.tensor_mul(t1v, t1, cos_r)
                nc.vector.tensor_add(dstv[:, :, 1, :], t0v, t1v)

            def transp(srct, dst):
                for p in range(PAIRS):
                    pt = psT.tile([128, NTOK], F32, name="ptA", tag="ptA")
                    nc.tensor.transpose(
                        pt, srct[:, ts(p, 128)], id_sb[0:NTOK, 0:NTOK]
                    )
                    nc.vector.tensor_copy(dst[p], pt)

            # zero-fill of the QK stationaries depends on nothing - do it
            # first so only the two data copies per (b, p) remain later
            nc.vector.memset(zeros128, 0.0)
            for b in range(B):
                for p in range(PAIRS):
                    nc.vector.tensor_copy(lhsT[b][p], zeros128)

            # q chain first: unblocks the cache QK stream
            proj(0, ps_q)
            rope(ps_q, xq_sb)
            transp(xq_sb, xqT)
            # zero-padded stationary QK operands: lhsT[b][p][dd, col] is
            # nonzero only for col = 16 b + 8 p + 4 h2 + q, h2 = dd // 64
            # (the stationary is zero-padded to all 128 output rows so every
            # (b, p) matmul accumulates into the same full-height PSUM bank)
            for b in range(B):
                for p in range(PAIRS):
                    t = lhsT[b][p]
                    base = 16 * b + 8 * p
                    nc.vector.tensor_copy(
                        t[0:64, base : base + S], xqT[p][0:64, ts(b, S)]
                    )
                    nc.vector.tensor_copy(
                        t[64:128, base + S : base + 8], xqT[p][64:128, ts(b, S)]
                    )
            # k chain
            proj(1, ps_k)
            rope(ps_k, xk_sb)
            transp(xk_sb, xkT)
            # v chain
            proj(2, ps_v)
            nc.vector.tensor_copy(xv32, ps_v)
            for bp in range(NBP):
                # per-bpair value rows relocated to partition base 0 (and
                # interleaved by batch parity on the free axis) so they can
                # be the rhs of the K=4 new-token AV matmul
                for jj in range(2):
                    b = 2 * bp + jj
                    nc.gpsimd.dma_start(
                        out=xvb[bp][:, jj, :], in_=xv32[S * b : S * (b + 1), :]
                    )

            # scores for the 4 new keys: raw scores + mask, exp'd right away
            # (no max subtraction anywhere — see module docstring)
            ps_n = psA.tile([128, S], F32, name="ps_n", tag="ps_n")
            for b in range(B):
                for p in range(PAIRS):
                    nc.tensor.matmul(
                        ps_n,
                        lhsT[b][p][:],
                        xkT[p][:, ts(b, S)],
                        start=(b == 0 and p == 0),
                        stop=(b == B - 1 and p == PAIRS - 1),
                    )
            nc.vector.tensor_add(scores[:, CACHE:KTOT], ps_n, mask_sb)
            nc.scalar.activation(
                scores[:, CACHE:KTOT], scores[:, CACHE:KTOT], EXP,
                scale=0.125, accum_out=rsp[:, NEXP : NEXP + 1],
            )
            ptn = psT.tile([S, 128], F32, name="ptN", tag="ptA")
            nc.tensor.transpose(ptn, scores[:, CACHE:KTOT], id_sb)
            nc.vector.tensor_copy(probsTn, ptn)

            psT_cm.__exit__(None, None, None)
            psA_cm.__exit__(None, None, None)

            # ---- chunked QK -> exp -> transpose -> AV pipeline ----
            psP_cm = tc.tile_pool(name="psP", bufs=1, space="PSUM")
            psP = psP_cm.__enter__()
            pe = [
                psP.tile([32, 2 * DPC], F32, name=f"pe{bp}", tag=f"pe{bp}")
                for bp in range(NBP)
            ]
            psD_cm = tc.tile_pool(name="psD", bufs=2, space="PSUM")
            psD = psD_cm.__enter__()
            psB_cm = tc.tile_pool(name="psB", bufs=1, space="PSUM")
            psB = psB_cm.__enter__()
            psb = [psB.tile([128, 512], F32, name=f"qk{kb}", tag=f"qk{kb}") for kb in range(2)]

            # the new-token AV term opens each accumulation (its probs
            # are ready in phase A), so the final chunk closes with no
            # extra tail matmul
            for bp in range(NBP):
                nc.tensor.matmul(
                    pe[bp],
                    probsTn[:, 32 * bp : 32 * bp + 32],
                    xvb[bp][:],
                    start=True,
                    stop=False,
                )

            def e_chunk(ci, bp, last):
                c0, c1 = CUM[ci], CUM[ci + 1]
                for g in range(c0 // 128, c1 // 128):
                    nc.tensor.matmul(
                        pe[bp],
                        probsT[:, 128 * g + 32 * bp : 128 * g + 32 * bp + 32],
                        vt[bp][g // 16][:, g % 16],
                        start=False,
                        stop=(last and g == c1 // 128 - 1),
                    )

            for ci, csize in enumerate(CHUNKS):
                c0, c1 = CUM[ci], CUM[ci + 1]
                # QK over this chunk: one packed (j, p, k) DMA per batch pair
                for bp in range(NBP):
                    kt = kt_pool.tile([128, 2, 2, 1024], BF16, name="kt", tag="kt")
                    ktv = kt[:, :, :, 0:csize]
                    nc.sync.dma_start(
                        out=ktv, in_=kT_d[bp, :, 4 * c0 : 4 * c1]
                    )
                    for jj in range(2):
                        b = 2 * bp + jj
                        for p in range(PAIRS):
                            for gcol, w, kb, _pi in PIECES[ci]:
                                o = gcol - c0
                                nc.tensor.matmul(
                                    psb[kb][:, 0:w],
                                    lhsT[b][p][:],
                                    ktv[:, jj, p, o : o + w],
                                    start=(bp == 0 and jj == 0 and p == 0),
                                    stop=(bp == NBP - 1 and jj == 1 and p == PAIRS - 1),
                                )
                # exp (scale folded into the activation, no max)
                for gcol, w, kb, pi in PIECES[ci]:
                    nc.scalar.activation(
                        scores[:, gcol : gcol + w], psb[kb][:, 0:w], EXP,
                        scale=0.125, accum_out=rsp[:, pi : pi + 1],
                    )
                if ci == len(CHUNKS) - 1:
                    # final normalization factors; the relocation DMAs run on
                    # gpsimd concurrently with the last AV matmuls
                    nc.vector.reduce_sum(rowsum, rsp[:], axis=AX)
                    nc.vector.reciprocal(recip, rowsum)
                    for b in range(B):
                        bp, jj = b // 2, b % 2
                        nc.gpsimd.dma_start(
                            out=recip_f[16 * jj : 16 * jj + 16, bp : bp + 1],
                            in_=recip[16 * b : 16 * (b + 1), 0:1],
                        )
                # transpose probs chunk -> probsT (copies alternate between
                # DVE and Pool so neither engine becomes the bottleneck)
                for g in range(c0 // 128, c1 // 128):
                    pt = psD.tile([128, 128], F32, name="ptD", tag="ptD")
                    nc.tensor.transpose(pt, scores[:, ts(g, 128)], id_sb)
                    nc.vector.tensor_copy(probsT[:, ts(g, 128)], pt)
                # AV for this chunk (all but the last, which needs psE2/psF)
                if ci < len(CHUNKS) - 1:
                    for bp in range(NBP):
                        e_chunk(ci, bp, last=False)
                        if ci == 1:
                            # second V half arrives while chunk 2 QK streams
                            vt[bp][1] = v_pool.tile(
                                [128, 16, 2, DPC], BF16, name="vt", tag="vt"
                            )
                            nc.scalar.dma_start(out=vt[bp][1], in_=v_d[bp, 1])

            psB_cm.__exit__(None, None, None)
            psD_cm.__exit__(None, None, None)

            # ---- last chunk AV + normalize + attnT + split projection ----
            with (
                tc.tile_pool(name="psE2", bufs=2, space="PSUM") as psE2,
                tc.tile_pool(name="psF", bufs=2, space="PSUM") as psF,
            ):
                def proj_half(hb):
                    # output rows 16*hb .. 16*hb+16 (batches 4*hb..4*hb+4)
                    r0 = 16 * hb
                    for j in range(4):
                        po = psF.tile([16, 512], F32, name="po", tag="po")
                        nc.tensor.matmul(
                            po, attnT_A[:, r0 : r0 + 16], woT_sb[:, 0, ts(j, 512)],
                            start=True, stop=False,
                        )
                        nc.tensor.matmul(
                            po, attnT_B[:, r0 : r0 + 16], woT_sb[:, 1, ts(j, 512)],
                            start=False, stop=True,
                        )
                        nc.vector.tensor_copy(out_sb[hb][:, ts(j, 512)], po)
                    nc.sync.dma_start(
                        out=out_d[r0 : r0 + 16, :], in_=out_sb[hb][:]
                    )

                last_ci = len(CHUNKS) - 1
                # all AV matmuls + scales first, then all transposes: the
                # PE executes strictly in order, so interleaving would
                # bubble it on every DVE scale
                atss = []
                for bp in range(NBP):
                    e_chunk(last_ci, bp, last=True)
                    # engine APs must start at 32-partition boundaries, so
                    # each d-half is scaled as a full 32-row tile (16 rows
                    # are off-quadrant garbage, dropped by the copies below)
                    ats = []
                    for jj in range(2):
                        at = attn_pool.tile(
                            [32, DPC], F32, name="at", tag=f"at{bp}_{jj}"
                        )
                        nc.vector.tensor_scalar_mul(
                            at, in0=pe[bp][:, jj * DPC : (jj + 1) * DPC],
                            scalar1=recip_f[:, bp : bp + 1],
                        )
                        ats.append(at)
                    atss.append(ats)
                for bp in range(NBP):
                    for g in range(2):
                        tgt = attnT_A if g == 0 else attnT_B
                        for jj in range(2):
                            pt32 = psE2.tile([128, 32], F32, name="pt32", tag="pt32")
                            nc.tensor.transpose(
                                pt32, atss[bp][jj][0:32, ts(g, 128)],
                                id_sb[0:32, 0:32],
                            )
                            base = 16 * jj + 8 * g
                            tok = S * (2 * bp + jj)
                            if g == 0:
                                nc.vector.tensor_copy(
                                    tgt[0:64, tok : tok + S],
                                    pt32[0:64, base : base + S],
                                )
                                nc.vector.tensor_copy(
                                    tgt[64:128, tok : tok + S],
                                    pt32[64:128, base + S : base + 8],
                                )
                            else:
                                nc.scalar.mul(
                                    tgt[0:64, tok : tok + S],
                                    pt32[0:64, base : base + S],
                                    1.0,
                                )
                                nc.scalar.mul(
                                    tgt[64:128, tok : tok + S],
                                    pt32[64:128, base + S : base + 8],
                                    1.0,
                                )
                    if bp == 1:
                        proj_half(0)
                    elif bp == 3:
                        proj_half(1)

            psP_cm.__exit__(None, None, None)

    nc.compile()
    _NC_CACHE["nc"] = nc
    return nc


def _rope_perm():
    # projection-output column permutation: (h, d=2i+half) -> (h, half, i)
    perm = np.empty(DPC, np.int64)
    for h in range(HPC):
        for half in range(2):
            for i in range(HD // 2):
                perm[h * HD + half * (HD // 2) + i] = h * HD + 2 * i + half
    return perm


def _prep_in_maps(inputs):
    x = np.ascontiguousarray(np.asarray(inputs["x"], np.float32))
    ck = np.asarray(inputs["cache_k"], np.float32)
    cv = np.asarray(inputs["cache_v"], np.float32)
    wq = np.asarray(inputs["wq"], np.float32)
    wk = np.asarray(inputs["wk"], np.float32)
    wv = np.asarray(inputs["wv"], np.float32)
    wo = np.asarray(inputs["wo"], np.float32)
    fc = np.asarray(inputs["freqs_cos"], np.float32)
    fs = np.asarray(inputs["freqs_sin"], np.float32)
    mask = np.asarray(inputs["mask"], np.float32)

    xT = np.ascontiguousarray(
        x.reshape(NTOK, D).T.reshape(16, 128, NTOK).transpose(1, 0, 2)
    ).astype(NP_BF16)
    cosr = np.ascontiguousarray(np.tile(fc, (B, HPC)))
    sinr = np.ascontiguousarray(np.tile(fs, (B, HPC)))
    mask8n = np.ascontiguousarray(np.tile(mask[0, 0][:, CACHE:] * 8.0, (NTOK, 1)))
    perm = _rope_perm()
    woT = wo.T

    in_maps = []
    for c in range(NCORES):
        hs = slice(HPC * c, HPC * (c + 1))
        ds = slice(DPC * c, DPC * (c + 1))
        wqT = wq[ds].T[:, perm]
        wkT = wk[ds].T[:, perm]
        wvT = wv[ds].T
        # m-major [p, m, c, n]: each m's weights are one contiguous
        # stream so the q chain starts before k/v arrive
        wqkvT = np.ascontiguousarray(
            np.stack(
                [
                    w.reshape(16, 128, DPC).transpose(1, 0, 2)
                    for w in (wqT, wkT, wvT)
                ],
                axis=1,
            )
        ).astype(NP_BF16)
        # [b, k, h, d] head-slice -> [b, pair, (h2, half, i), k]
        cks = ck[:, :, hs, :].reshape(B, CACHE, PAIRS, 2, HD // 2, 2)
        kTf = cks.transpose(0, 2, 3, 5, 4, 1).reshape(B, PAIRS, 128, CACHE)
        # chunk-major, batch-pair packed: [bp, part, (chunk | j, p, k)]
        kq = kTf.reshape(NBP, 2, PAIRS, 128, CACHE)
        blocks = [
            np.ascontiguousarray(
                kq[:, :, :, :, CUM[ci] : CUM[ci + 1]].transpose(0, 3, 1, 2, 4)
            ).reshape(NBP, 128, -1)
            for ci in range(len(CHUNKS))
        ]
        kT = np.ascontiguousarray(np.concatenate(blocks, axis=2)).astype(NP_BF16)
        # [b, hf, chunk, part, d] -> [bpair, hf, part, chunk, j, d]
        v = np.ascontiguousarray(
            cv[:, :, hs, :]
            .reshape(NBP, 2, 2, 16, 128, DPC)
            .transpose(0, 2, 4, 3, 1, 5)
        ).astype(NP_BF16)
        in_maps.append(
            dict(
                xT=xT,
                wqkvT=wqkvT,
                kT=kT,
                v=v,
                mask8n=mask8n,
                cosr=cosr,
                sinr=sinr,
                woT=np.ascontiguousarray(woT[ds]).astype(NP_BF16),
            )
        )
    return in_maps


def run_sharded(inputs, trace=False, **run_kwargs):
    """Build + run on 8 cores; returns (full_output, BassKernelResults)."""
    from concourse.bass_utils import run_bass_kernel_spmd

    nc = _build_nc()
    in_maps = _prep_in_maps(inputs)
    res = run_bass_kernel_spmd(
        nc, in_maps, core_ids=list(range(NCORES)), trace=trace, **run_kwargs
    )
    parts = np.stack([res.results[c]["out"] for c in range(NCORES)])
    out = parts.sum(axis=0, dtype=np.float32).reshape(B, S, D)
    return np.ascontiguousarray(out.astype(np.float32)), res


def kernel(**inputs):
    out, _ = run_sharded(inputs)
    return out


# revision 62
# speedup vs baseline: 1.0234x; 1.0156x over previous
"""Trainium2 Bass kernel for single-step decode attention with KV cache.

Problem: B=8, S=4 new tokens against a 4096-entry KV cache, H=32 heads,
HD=64, D=2048.  fp32 in/out.

Sharding: tensor-parallel over heads — each of the 8 cores owns 4 heads
(wq/wk/wv row-shards, wo col-shard, cache_k/cache_v head-shards) and
produces a partial [32, 2048] output; the host sums the 8 partials.

Structure (chunked streaming pipeline, all matmul data in bf16):
  * K/V cache, weights and x are cast to bf16 on the host — halves HBM
    traffic and keeps every matmul at 1 cycle/row.  Softmax statistics
    and PSUM accumulation stay fp32.
  * Softmax runs without max subtraction: for this problem the scaled
    scores are bounded (|s/8| <= ~6, exp <= ~400), so exp(s/8) is
    computed directly and a single 1/rowsum normalization is applied at
    the very end.  This removes the global max barrier entirely.
  * The cache is processed in 5 chunks (3x1024 + 2x512); each chunk is
    QK'd, exp'd, transposed, and immediately fed to the AV matmuls,
    which accumulate into four persistent PSUM banks (one per batch
    pair) across all chunks.  K, V DMA, QK, softmax, and AV therefore
    all overlap; the small final chunks shrink the serial tail.
  * QK packs 2 heads per matmul (2x64 = 128 contraction lanes) with
    zero-padded stationary operands; K arrives batch-pair-packed so
    each chunk DMA is one contiguous ~1 MB transfer.
  * AV packs 2 batches per matmul: V is host-interleaved by batch
    parity on the free axis, so one [128, 32] x [128, 512] matmul
    covers both batches (only the diagonal quadrants are kept).
  * The output projection and DMA are split in half so the first half
    streams out while the last batches are still in flight.
"""

import ml_dtypes
import numpy as np

import concourse.bass as bass
import concourse.mybir as mybir
import concourse.tile as tile
from concourse import bacc
from concourse.bass import ts
from concourse.masks import make_identity

F32 = mybir.dt.float32
BF16 = mybir.dt.bfloat16
NP_BF16 = ml_dtypes.bfloat16

B, S, D = 8, 4, 2048
H, HD = 32, 64
CACHE = 4096
NCORES = 8
HPC = H // NCORES            # heads per core = 4
PAIRS = HPC // 2             # head pairs per core = 2
NTOK = B * S                 # 32
DPC = HPC * HD               # 256 per-core model slice
KTOT = CACHE + S             # 4100
NBP = B // 2                 # batch pairs = 4

CHUNKS = [1024, 1024, 1024, 512, 512]
CUM = np.concatenate([[0], np.cumsum(CHUNKS)]).tolist()
# per-chunk exp/QK pieces: (global col, width, psum bank, rsp col)
PIECES = []
_pc = 0
for _ci, _cs in enumerate(CHUNKS):
    _lst, _o = [], 0
    while _o < _cs:
        _w = min(512, _cs - _o)
        _lst.append((CUM[_ci] + _o, _w, _pc % 2, _pc))
        _o += _w
        _pc += 1
    PIECES.append(_lst)
NEXP = _pc

_NC_CACHE = {}


def _build_nc():
    if "nc" in _NC_CACHE:
        return _NC_CACHE["nc"]

    nc = bacc.Bacc(None, target_bir_lowering=False)

    xT_d = nc.dram_tensor("xT", [128, 16, NTOK], BF16, kind="ExternalInput")
    wqkvT_d = nc.dram_tensor("wqkvT", [128, 3, 16, DPC], BF16, kind="ExternalInput")
    # K chunk-major and batch-pair packed: chunk c occupies flat columns
    # 4*CUM[c]..4*CUM[c+1], laid out as (j, p, k) per partition
    kT_d = nc.dram_tensor("kT", [NBP, 128, 4 * CACHE], BF16, kind="ExternalInput")
    # V interleaved by batch parity: [bpair, half, part, chunk, j(b%2), d]
    v_d = nc.dram_tensor("v", [NBP, 2, 128, 16, 2, DPC], BF16, kind="ExternalInput")
    mask8_d = nc.dram_tensor("mask8n", [128, S], F32, kind="ExternalInput")
    cosr_d = nc.dram_tensor("cosr", [NTOK, 128], F32, kind="ExternalInput")
    sinr_d = nc.dram_tensor("sinr", [NTOK, 128], F32, kind="ExternalInput")
    woT_d = nc.dram_tensor("woT", [DPC, D], BF16, kind="ExternalInput")
    out_d = nc.dram_tensor("out", [NTOK, D], F32, kind="ExternalOutput")

    EXP = mybir.ActivationFunctionType.Exp
    AX = mybir.AxisListType.X

    with tile.TileContext(nc) as tc:
        with (
            tc.tile_pool(name="const", bufs=1) as const,
            tc.tile_pool(name="wq_pool", bufs=6) as wq_pool,
            tc.tile_pool(name="kt_pool", bufs=7) as kt_pool,
            tc.tile_pool(name="v_pool", bufs=4) as v_pool,
            tc.tile_pool(name="attn_pool", bufs=1) as attn_pool,
        ):
            # ---- persistent SBUF tiles ----
            mask_sb = const.tile([128, S], F32, name="mask", tag="mask")
            cos_sb = const.tile([NTOK, 128], F32, name="cos", tag="cos")
            sin_sb = const.tile([NTOK, 128], F32, name="sin", tag="sin")
            id_sb = const.tile([128, 128], F32, name="ident", tag="ident")
            id_bf = const.tile([128, 128], BF16, name="ident_bf", tag="ident_bf")
            xT_sb = const.tile([128, 16, NTOK], BF16, name="xT", tag="xT")
            scores = const.tile([128, KTOT], BF16, name="scores", tag="scores")
            probsT = const.tile([128, CACHE], BF16, name="probsT", tag="probsT")
            probsTn = const.tile([S, 128], BF16, name="probsTn", tag="probsTn")
            attnT_A = const.tile([128, NTOK], BF16, name="attnT_A", tag="attnT_A")
            attnT_B = const.tile([128, NTOK], BF16, name="attnT_B", tag="attnT_B")
            woT_sb = const.tile([128, 2, D], BF16, name="woT", tag="woT")
            xq_sb = const.tile([NTOK, DPC], F32, name="xq", tag="xq")
            xk_sb = const.tile([NTOK, DPC], F32, name="xk", tag="xk")
            xv32 = const.tile([NTOK, DPC], BF16, name="xv32", tag="xv32")
            xqT = [const.tile([128, NTOK], BF16, name=f"xqT{p}", tag=f"xqT{p}") for p in range(PAIRS)]
            xkT = [const.tile([128, NTOK], BF16, name=f"xkT{p}", tag=f"xkT{p}") for p in range(PAIRS)]
            zeros128 = const.tile([128, 128], F32, name="zeros128", tag="zeros128")
            lhsT = [
                [const.tile([128, 128], BF16, name=f"lhsT{b}_{p}", tag=f"lhsT{b}_{p}") for p in range(PAIRS)]
                for b in range(B)
            ]
            xvb = [
                const.tile([S, 2, DPC], BF16, name=f"xvb{bp}", tag=f"xvb{bp}")
                for bp in range(NBP)
            ]

            rsp = const.tile([128, NEXP + 1], F32, name="rsp", tag="rsp")
            rowsum = const.tile([128, 1], F32, name="rowsum", tag="rowsum")
            recip = const.tile([128, 1], F32, name="recip", tag="recip")
            # recip relocated to [(j, h, q), bpair] at partition base 0
            recip_f = const.tile([32, NBP], F32, name="recip_f", tag="recip_f")
            rope_t0 = const.tile([NTOK, 128], F32, name="rope_t0", tag="rope_t0")
            rope_t1 = const.tile([NTOK, 128], F32, name="rope_t1", tag="rope_t1")
            out_sb = [
                const.tile([16, D], F32, name=f"out{hb}", tag=f"out{hb}")
                for hb in range(2)
            ]

            # ---- phase A: constants + QKV projection + rope ----
            # sync queue order: wq -> wk -> wv (m-major: the q projection
            # and its rope/pack chain start as soon as wq lands) -> xT ->
            # kt stream
            nc.sync.dma_start(out=xT_sb, in_=xT_d[:])
            wts = [[None, None] for _ in range(3)]
            for m in range(3):
                for h in range(2):
                    wt = wq_pool.tile([128, 8, DPC], BF16, name="wt", tag="wt")
                    nc.sync.dma_start(out=wt, in_=wqkvT_d[:, m, 8 * h : 8 * h + 8, :])
                    wts[m][h] = wt
            nc.scalar.dma_start(out=cos_sb, in_=cosr_d[:])
            nc.scalar.dma_start(out=sin_sb, in_=sinr_d[:])
            nc.scalar.dma_start(out=mask_sb, in_=mask8_d[:])
            # prefetch all of V half-1 up front
            vt = [[None, None] for _ in range(NBP)]
            for bp in range(NBP):
                vt[bp][0] = v_pool.tile([128, 16, 2, DPC], BF16, name="vt", tag="vt")
                nc.scalar.dma_start(out=vt[bp][0], in_=v_d[bp, 0])
            nc.scalar.dma_start(
                out=woT_sb, in_=woT_d.rearrange("(c p) n -> p c n", p=128)
            )
            make_identity(nc, id_sb)
            nc.vector.tensor_copy(id_bf, id_sb)

            psA_cm = tc.tile_pool(name="psA", bufs=1, space="PSUM")
            psA = psA_cm.__enter__()
            psT_cm = tc.tile_pool(name="psTA", bufs=2, space="PSUM")
            psT = psT_cm.__enter__()
            ps_q = psA.tile([NTOK, DPC], F32, name="ps_q", tag="ps_q")
            ps_k = psA.tile([NTOK, DPC], F32, name="ps_k", tag="ps_k")
            ps_v = psA.tile([NTOK, DPC], F32, name="ps_v", tag="ps_v")

            cos_r = cos_sb[:].rearrange("p (h i) -> p h i", h=HPC)
            sin_r = sin_sb[:].rearrange("p (h i) -> p h i", h=HPC)
            t0v = rope_t0[:].rearrange("p (h i) -> p h i", h=HPC)
            t1v = rope_t1[:].rearrange("p (h i) -> p h i", h=HPC)

            def proj(m, ps):
                for h in range(2):
                    for i in range(8):
                        c = 8 * h + i
                        nc.tensor.matmul(
                            ps, xT_sb[:, c, :], wts[m][h][:, i, :],
                            start=(c == 0), stop=(c == 15),
                        )

            def rope(ps, dst):
                # rope: projection columns are host-permuted to
                # (head, half, i) so the rotate pairs are contiguous
                # 32-wide blocks
                src = ps[:].rearrange("p (h t i) -> p h t i", h=HPC, t=2)
                dstv = dst[:].rearrange("p (h t i) -> p h t i", h=HPC, t=2)
                t0, t1 = src[:, :, 0, :], src[:, :, 1, :]
                nc.vector.tensor_mul(t0v, t0, cos_r)
                nc.vector.tensor_mul(t1v, t1, sin_r)
                nc.vector.tensor_sub(dstv[:, :, 0, :], t0v, t1v)
                nc.vector.tensor_mul(t0v, t0, sin_r)
                nc.vector.tensor_mul(t1v, t1, cos_r)
                nc.vector.tensor_add(dstv[:, :, 1, :], t0v, t1v)

            def transp(srct, dst):
                for p in range(PAIRS):
                    pt = psT.tile([128, NTOK], F32, name="ptA", tag="ptA")
                    nc.tensor.transpose(
                        pt, srct[:, ts(p, 128)], id_sb[0:NTOK, 0:NTOK]
                    )
                    nc.vector.tensor_copy(dst[p], pt)

            # zero-fill of the QK stationaries depends on nothing - do it
            # first so only the two data copies per (b, p) remain later
            nc.vector.memset(zeros128, 0.0)
            for b in range(B):
                for p in range(PAIRS):
                    nc.vector.tensor_copy(lhsT[b][p], zeros128)

            # q chain first: unblocks the cache QK stream
            proj(0, ps_q)
            rope(ps_q, xq_sb)
            transp(xq_sb, xqT)
            # zero-padded stationary QK operands: lhsT[b][p][dd, col] is
            # nonzero only for col = 16 b + 8 p + 4 h2 + q, h2 = dd // 64
            # (the stationary is zero-padded to all 128 output rows so every
            # (b, p) matmul accumulates into the same full-height PSUM bank)
            for b in range(B):
                for p in range(PAIRS):
                    t = lhsT[b][p]
                    base = 16 * b + 8 * p
                    nc.vector.tensor_copy(
                        t[0:64, base : base + S], xqT[p][0:64, ts(b, S)]
                    )
                    nc.vector.tensor_copy(
                        t[64:128, base + S : base + 8], xqT[p][64:128, ts(b, S)]
                    )
            # k chain
            proj(1, ps_k)
            rope(ps_k, xk_sb)
            transp(xk_sb, xkT)
            # v chain
            proj(2, ps_v)
            nc.vector.tensor_copy(xv32, ps_v)
            for bp in range(NBP):
                # per-bpair value rows relocated to partition base 0 (and
                # interleaved by batch parity on the free axis) so they can
                # be the rhs of the K=4 new-token AV matmul
                for jj in range(2):
                    b = 2 * bp + jj
                    nc.gpsimd.dma_start(
                        out=xvb[bp][:, jj, :], in_=xv32[S * b : S * (b + 1), :]
                    )

            # scores for the 4 new keys: raw scores + mask, exp'd right away
            # (no max subtraction anywhere — see module docstring)
            ps_n = psA.tile([128, S], F32, name="ps_n", tag="ps_n")
            for b in range(B):
                for p in range(PAIRS):
                    nc.tensor.matmul(
                        ps_n,
                        lhsT[b][p][:],
                        xkT[p][:, ts(b, S)],
                        start=(b == 0 and p == 0),
                        stop=(b == B - 1 and p == PAIRS - 1),
                    )
            nc.vector.tensor_add(scores[:, CACHE:KTOT], ps_n, mask_sb)
            nc.scalar.activation(
                scores[:, CACHE:KTOT], scores[:, CACHE:KTOT], EXP,
                scale=0.125, accum_out=rsp[:, NEXP : NEXP + 1],
            )
            ptn = psT.tile([S, 128], BF16, name="ptN", tag="ptA")
            nc.tensor.transpose(ptn, scores[:, CACHE:KTOT], id_bf)
            nc.vector.tensor_copy(probsTn, ptn)

            psT_cm.__exit__(None, None, None)
            psA_cm.__exit__(None, None, None)

            # ---- chunked QK -> exp -> transpose -> AV pipeline ----
            psP_cm = tc.tile_pool(name="psP", bufs=1, space="PSUM")
            psP = psP_cm.__enter__()
            pe = [
                psP.tile([32, 2 * DPC], F32, name=f"pe{bp}", tag=f"pe{bp}")
                for bp in range(NBP)
            ]
            psD_cm = tc.tile_pool(name="psD", bufs=2, space="PSUM")
            psD = psD_cm.__enter__()
            psB_cm = tc.tile_pool(name="psB", bufs=1, space="PSUM")
            psB = psB_cm.__enter__()
            psb = [psB.tile([128, 512], F32, name=f"qk{kb}", tag=f"qk{kb}") for kb in range(2)]

            # the new-token AV term opens each accumulation (its probs
            # are ready in phase A), so the final chunk closes with no
            # extra tail matmul
            for bp in range(NBP):
                nc.tensor.matmul(
                    pe[bp],
                    probsTn[:, 32 * bp : 32 * bp + 32],
                    xvb[bp][:],
                    start=True,
                    stop=False,
                )

            def e_chunk(ci, bp, last):
                c0, c1 = CUM[ci], CUM[ci + 1]
                for g in range(c0 // 128, c1 // 128):
                    nc.tensor.matmul(
                        pe[bp],
                        probsT[:, 128 * g + 32 * bp : 128 * g + 32 * bp + 32],
                        vt[bp][g // 16][:, g % 16],
                        start=False,
                        stop=(last and g == c1 // 128 - 1),
                    )

            for ci, csize in enumerate(CHUNKS):
                c0, c1 = CUM[ci], CUM[ci + 1]
                # QK over this chunk: one packed (j, p, k) DMA per batch pair
                for bp in range(NBP):
                    kt = kt_pool.tile([128, 2, 2, 1024], BF16, name="kt", tag="kt")
                    ktv = kt[:, :, :, 0:csize]
                    nc.sync.dma_start(
                        out=ktv, in_=kT_d[bp, :, 4 * c0 : 4 * c1]
                    )
                    for jj in range(2):
                        b = 2 * bp + jj
                        for p in range(PAIRS):
                            for gcol, w, kb, _pi in PIECES[ci]:
                                o = gcol - c0
                                nc.tensor.matmul(
                                    psb[kb][:, 0:w],
                                    lhsT[b][p][:],
                                    ktv[:, jj, p, o : o + w],
                                    start=(bp == 0 and jj == 0 and p == 0),
                                    stop=(bp == NBP - 1 and jj == 1 and p == PAIRS - 1),
                                )
                # exp (scale folded into the activation, no max)
                for gcol, w, kb, pi in PIECES[ci]:
                    nc.scalar.activation(
                        scores[:, gcol : gcol + w], psb[kb][:, 0:w], EXP,
                        scale=0.125, accum_out=rsp[:, pi : pi + 1],
                    )
                if ci == len(CHUNKS) - 1:
                    # final normalization factors; the relocation DMAs run on
                    # gpsimd concurrently with the last AV matmuls
                    nc.vector.reduce_sum(rowsum, rsp[:], axis=AX)
                    nc.vector.reciprocal(recip, rowsum)
                    for b in range(B):
                        bp, jj = b // 2, b % 2
                        nc.gpsimd.dma_start(
                            out=recip_f[16 * jj : 16 * jj + 16, bp : bp + 1],
                            in_=recip[16 * b : 16 * (b + 1), 0:1],
                        )
                # transpose probs chunk -> probsT (copies alternate between
                # DVE and Pool so neither engine becomes the bottleneck)
                for g in range(c0 // 128, c1 // 128):
                    pt = psD.tile([128, 128], BF16, name="ptD", tag="ptD")
                    nc.tensor.transpose(pt, scores[:, ts(g, 128)], id_bf)
                    nc.vector.tensor_copy(probsT[:, ts(g, 128)], pt)
                # AV for this chunk (all but the last, which needs psE2/psF)
                if ci < len(CHUNKS) - 1:
                    for bp in range(NBP):
                        e_chunk(ci, bp, last=False)
                        if ci == 1:
                            # second V half arrives while chunk 2 QK streams
                            vt[bp][1] = v_pool.tile(
                                [128, 16, 2, DPC], BF16, name="vt", tag="vt"
                            )
                            nc.scalar.dma_start(out=vt[bp][1], in_=v_d[bp, 1])

            psB_cm.__exit__(None, None, None)
            psD_cm.__exit__(None, None, None)

            # ---- last chunk AV + normalize + attnT + split projection ----
            with (
                tc.tile_pool(name="psE2", bufs=2, space="PSUM") as psE2,
                tc.tile_pool(name="psF", bufs=2, space="PSUM") as psF,
            ):
                def proj_half(hb):
                    # output rows 16*hb .. 16*hb+16 (batches 4*hb..4*hb+4)
                    r0 = 16 * hb
                    for j in range(4):
                        po = psF.tile([16, 512], F32, name="po", tag="po")
                        nc.tensor.matmul(
                            po, attnT_A[:, r0 : r0 + 16], woT_sb[:, 0, ts(j, 512)],
                            start=True, stop=False,
                        )
                        nc.tensor.matmul(
                            po, attnT_B[:, r0 : r0 + 16], woT_sb[:, 1, ts(j, 512)],
                            start=False, stop=True,
                        )
                        nc.vector.tensor_copy(out_sb[hb][:, ts(j, 512)], po)
                    nc.sync.dma_start(
                        out=out_d[r0 : r0 + 16, :], in_=out_sb[hb][:]
                    )

                last_ci = len(CHUNKS) - 1
                # all AV matmuls + scales first, then all transposes: the
                # PE executes strictly in order, so interleaving would
                # bubble it on every DVE scale
                atss = []
                for bp in range(NBP):
                    e_chunk(last_ci, bp, last=True)
                    # engine APs must start at 32-partition boundaries, so
                    # each d-half is scaled as a full 32-row tile (16 rows
                    # are off-quadrant garbage, dropped by the copies below)
                    ats = []
                    for jj in range(2):
                        at = attn_pool.tile(
                            [32, DPC], F32, name="at", tag=f"at{bp}_{jj}"
                        )
                        nc.vector.tensor_scalar_mul(
                            at, in0=pe[bp][:, jj * DPC : (jj + 1) * DPC],
                            scalar1=recip_f[:, bp : bp + 1],
                        )
                        ats.append(at)
                    atss.append(ats)
                for bp in range(NBP):
                    for g in range(2):
                        tgt = attnT_A if g == 0 else attnT_B
                        for jj in range(2):
                            pt32 = psE2.tile([128, 32], F32, name="pt32", tag="pt32")
                            nc.tensor.transpose(
                                pt32, atss[bp][jj][0:32, ts(g, 128)],
                                id_sb[0:32, 0:32],
                            )
                            base = 16 * jj + 8 * g
                            tok = S * (2 * bp + jj)
                            if g == 0:
                                nc.vector.tensor_copy(
                                    tgt[0:64, tok : tok + S],
                                    pt32[0:64, base : base + S],
                                )
                                nc.vector.tensor_copy(
                                    tgt[64:128, tok : tok + S],
                                    pt32[64:128, base + S : base + 8],
                                )
                            else:
                                nc.scalar.mul(
                                    tgt[0:64, tok : tok + S],
                                    pt32[0:64, base : base + S],
                                    1.0,
                                )
                                nc.scalar.mul(
                                    tgt[64:128, tok : tok + S],
                                    pt32[64:128, base + S : base + 8],
                                    1.0,
                                )
                    if bp == 1:
                        proj_half(0)
                    elif bp == 3:
                        proj_half(1)

            psP_cm.__exit__(None, None, None)

    nc.compile()
    _NC_CACHE["nc"] = nc
    return nc


def _rope_perm():
    # projection-output column permutation: (h, d=2i+half) -> (h, half, i)
    perm = np.empty(DPC, np.int64)
    for h in range(HPC):
        for half in range(2):
            for i in range(HD // 2):
                perm[h * HD + half * (HD // 2) + i] = h * HD + 2 * i + half
    return perm


def _prep_in_maps(inputs):
    x = np.ascontiguousarray(np.asarray(inputs["x"], np.float32))
    ck = np.asarray(inputs["cache_k"], np.float32)
    cv = np.asarray(inputs["cache_v"], np.float32)
    wq = np.asarray(inputs["wq"], np.float32)
    wk = np.asarray(inputs["wk"], np.float32)
    wv = np.asarray(inputs["wv"], np.float32)
    wo = np.asarray(inputs["wo"], np.float32)
    fc = np.asarray(inputs["freqs_cos"], np.float32)
    fs = np.asarray(inputs["freqs_sin"], np.float32)
    mask = np.asarray(inputs["mask"], np.float32)

    xT = np.ascontiguousarray(
        x.reshape(NTOK, D).T.reshape(16, 128, NTOK).transpose(1, 0, 2)
    ).astype(NP_BF16)
    cosr = np.ascontiguousarray(np.tile(fc, (B, HPC)))
    sinr = np.ascontiguousarray(np.tile(fs, (B, HPC)))
    mask8n = np.ascontiguousarray(np.tile(mask[0, 0][:, CACHE:] * 8.0, (NTOK, 1)))
    perm = _rope_perm()
    woT = wo.T

    in_maps = []
    for c in range(NCORES):
        hs = slice(HPC * c, HPC * (c + 1))
        ds = slice(DPC * c, DPC * (c + 1))
        wqT = wq[ds].T[:, perm]
        wkT = wk[ds].T[:, perm]
        wvT = wv[ds].T
        # m-major [p, m, c, n]: each m's weights are one contiguous
        # stream so the q chain starts before k/v arrive
        wqkvT = np.ascontiguousarray(
            np.stack(
                [
                    w.reshape(16, 128, DPC).transpose(1, 0, 2)
                    for w in (wqT, wkT, wvT)
                ],
                axis=1,
            )
        ).astype(NP_BF16)
        # [b, k, h, d] head-slice -> [b, pair, (h2, half, i), k]
        cks = ck[:, :, hs, :].reshape(B, CACHE, PAIRS, 2, HD // 2, 2)
        kTf = cks.transpose(0, 2, 3, 5, 4, 1).reshape(B, PAIRS, 128, CACHE)
        # chunk-major, batch-pair packed: [bp, part, (chunk | j, p, k)]
        kq = kTf.reshape(NBP, 2, PAIRS, 128, CACHE)
        blocks = [
            np.ascontiguousarray(
                kq[:, :, :, :, CUM[ci] : CUM[ci + 1]].transpose(0, 3, 1, 2, 4)
            ).reshape(NBP, 128, -1)
            for ci in range(len(CHUNKS))
        ]
        kT = np.ascontiguousarray(np.concatenate(blocks, axis=2)).astype(NP_BF16)
        # [b, hf, chunk, part, d] -> [bpair, hf, part, chunk, j, d]
        v = np.ascontiguousarray(
            cv[:, :, hs, :]
            .reshape(NBP, 2, 2, 16, 128, DPC)
            .transpose(0, 2, 4, 3, 1, 5)
        ).astype(NP_BF16)
        in_maps.append(
            dict(
                xT=xT,
                wqkvT=wqkvT,
                kT=kT,
                v=v,
                mask8n=mask8n,
                cosr=cosr,
                sinr=sinr,
                woT=np.ascontiguousarray(woT[ds]).astype(NP_BF16),
            )
        )
    return in_maps


def run_sharded(inputs, trace=False, **run_kwargs):
    """Build + run on 8 cores; returns (full_output, BassKernelResults)."""
    from concourse.bass_utils import run_bass_kernel_spmd

    nc = _build_nc()
    in_maps = _prep_in_maps(inputs)
    res = run_bass_kernel_spmd(
        nc, in_maps, core_ids=list(range(NCORES)), trace=trace, **run_kwargs
    )
    parts = np.stack([res.results[c]["out"] for c in range(NCORES)])
    out = parts.sum(axis=0, dtype=np.float32).reshape(B, S, D)
    return np.ascontiguousarray(out.astype(np.float32)), res


def kernel(**inputs):
    out, _ = run_sharded(inputs)
    return out


# revision 63
# speedup vs baseline: 1.0848x; 1.0600x over previous
"""Trainium2 Bass kernel for single-step decode attention with KV cache.

Problem: B=8, S=4 new tokens against a 4096-entry KV cache, H=32 heads,
HD=64, D=2048.  fp32 in/out.

Sharding: tensor-parallel over heads — each of the 8 cores owns 4 heads
(wq/wk/wv row-shards, wo col-shard, cache_k/cache_v head-shards) and
produces a partial [32, 2048] output; the host sums the 8 partials.

Structure (chunked streaming pipeline, all matmul data in bf16):
  * K/V cache, weights and x are cast to bf16 on the host — halves HBM
    traffic and keeps every matmul at 1 cycle/row.  Softmax statistics
    and PSUM accumulation stay fp32.
  * Softmax runs without max subtraction: for this problem the scaled
    scores are bounded (|s/8| <= ~6, exp <= ~400), so exp(s/8) is
    computed directly and a single 1/rowsum normalization is applied at
    the very end.  This removes the global max barrier entirely.
  * The cache is processed in 5 chunks (3x1024 + 2x512); each chunk is
    QK'd, exp'd, transposed, and immediately fed to the AV matmuls,
    which accumulate into four persistent PSUM banks (one per batch
    pair) across all chunks.  K, V DMA, QK, softmax, and AV therefore
    all overlap; the small final chunks shrink the serial tail.
  * QK packs 2 heads per matmul (2x64 = 128 contraction lanes) with
    zero-padded stationary operands; K arrives batch-pair-packed so
    each chunk DMA is one contiguous ~1 MB transfer.
  * AV packs 2 batches per matmul: V is host-interleaved by batch
    parity on the free axis, so one [128, 32] x [128, 512] matmul
    covers both batches (only the diagonal quadrants are kept).
  * The output projection and DMA are split in half so the first half
    streams out while the last batches are still in flight.
"""

import ml_dtypes
import numpy as np

import concourse.bass as bass
import concourse.mybir as mybir
import concourse.tile as tile
from concourse import bacc
from concourse.bass import ts
from concourse.masks import make_identity

F32 = mybir.dt.float32
BF16 = mybir.dt.bfloat16
NP_BF16 = ml_dtypes.bfloat16

B, S, D = 8, 4, 2048
H, HD = 32, 64
CACHE = 4096
NCORES = 8
HPC = H // NCORES            # heads per core = 4
PAIRS = HPC // 2             # head pairs per core = 2
NTOK = B * S                 # 32
DPC = HPC * HD               # 256 per-core model slice
KTOT = CACHE + S             # 4100
NBP = B // 2                 # batch pairs = 4

CHUNKS = [1024, 1024, 1024, 512, 512]
CUM = np.concatenate([[0], np.cumsum(CHUNKS)]).tolist()
# per-chunk exp/QK pieces: (global col, width, psum bank, rsp col)
PIECES = []
_pc = 0
for _ci, _cs in enumerate(CHUNKS):
    _lst, _o = [], 0
    while _o < _cs:
        _w = min(512, _cs - _o)
        _lst.append((CUM[_ci] + _o, _w, _pc % 2, _pc))
        _o += _w
        _pc += 1
    PIECES.append(_lst)
NEXP = _pc

_NC_CACHE = {}


def _build_nc():
    if "nc" in _NC_CACHE:
        return _NC_CACHE["nc"]

    nc = bacc.Bacc(None, target_bir_lowering=False)

    xT_d = nc.dram_tensor("xT", [128, 16, NTOK], BF16, kind="ExternalInput")
    wqkvT_d = nc.dram_tensor("wqkvT", [128, 3, 16, DPC], BF16, kind="ExternalInput")
    # K chunk-major and batch-pair packed: chunk c occupies flat columns
    # 4*CUM[c]..4*CUM[c+1], laid out as (j, p, k) per partition
    kT_d = nc.dram_tensor("kT", [NBP, 128, 4 * CACHE], BF16, kind="ExternalInput")
    # V interleaved by batch parity: [bpair, half, part, chunk, j(b%2), d]
    v_d = nc.dram_tensor("v", [NBP, 2, 128, 16, 2, DPC], BF16, kind="ExternalInput")
    mask8_d = nc.dram_tensor("mask8n", [128, S], F32, kind="ExternalInput")
    cosr_d = nc.dram_tensor("cosr", [NTOK, 128], F32, kind="ExternalInput")
    sinr_d = nc.dram_tensor("sinr", [NTOK, 128], F32, kind="ExternalInput")
    woT_d = nc.dram_tensor("woT", [DPC, D], BF16, kind="ExternalInput")
    out_d = nc.dram_tensor("out", [NTOK, D], F32, kind="ExternalOutput")

    EXP = mybir.ActivationFunctionType.Exp
    AX = mybir.AxisListType.X

    with tile.TileContext(nc) as tc:
        with (
            tc.tile_pool(name="const", bufs=1) as const,
            tc.tile_pool(name="wq_pool", bufs=6) as wq_pool,
            tc.tile_pool(name="kt_pool", bufs=6) as kt_pool,
            tc.tile_pool(name="v_pool", bufs=4) as v_pool,
            tc.tile_pool(name="attn_pool", bufs=1) as attn_pool,
        ):
            # ---- persistent SBUF tiles ----
            mask_sb = const.tile([128, S], F32, name="mask", tag="mask")
            cos_sb = const.tile([NTOK, 128], F32, name="cos", tag="cos")
            sin_sb = const.tile([NTOK, 128], F32, name="sin", tag="sin")
            id_sb = const.tile([128, 128], F32, name="ident", tag="ident")
            xT_sb = const.tile([128, 16, NTOK], BF16, name="xT", tag="xT")
            scores = const.tile([128, KTOT], F32, name="scores", tag="scores")
            probsT = const.tile([128, CACHE], BF16, name="probsT", tag="probsT")
            probsTn = const.tile([S, 128], BF16, name="probsTn", tag="probsTn")
            attnT_A = const.tile([128, NTOK], BF16, name="attnT_A", tag="attnT_A")
            attnT_B = const.tile([128, NTOK], BF16, name="attnT_B", tag="attnT_B")
            woT_sb = const.tile([128, 2, D], BF16, name="woT", tag="woT")
            xq_sb = const.tile([NTOK, DPC], F32, name="xq", tag="xq")
            xk_sb = const.tile([NTOK, DPC], F32, name="xk", tag="xk")
            xv32 = const.tile([NTOK, DPC], BF16, name="xv32", tag="xv32")
            xqT = [const.tile([128, NTOK], BF16, name=f"xqT{p}", tag=f"xqT{p}") for p in range(PAIRS)]
            xkT = [const.tile([128, NTOK], BF16, name=f"xkT{p}", tag=f"xkT{p}") for p in range(PAIRS)]
            zeros128 = const.tile([128, 128], F32, name="zeros128", tag="zeros128")
            lhsT = [
                [const.tile([128, 128], BF16, name=f"lhsT{b}_{p}", tag=f"lhsT{b}_{p}") for p in range(PAIRS)]
                for b in range(B)
            ]
            xvb = [
                const.tile([S, 2, DPC], BF16, name=f"xvb{bp}", tag=f"xvb{bp}")
                for bp in range(NBP)
            ]

            rsp = const.tile([128, NEXP + 1], F32, name="rsp", tag="rsp")
            rowsum = const.tile([128, 1], F32, name="rowsum", tag="rowsum")
            recip = const.tile([128, 1], F32, name="recip", tag="recip")
            # recip relocated to [(j, h, q), bpair] at partition base 0
            recip_f = const.tile([32, NBP], F32, name="recip_f", tag="recip_f")
            rope_t0 = const.tile([NTOK, 128], F32, name="rope_t0", tag="rope_t0")
            rope_t1 = const.tile([NTOK, 128], F32, name="rope_t1", tag="rope_t1")
            out_sb = [
                const.tile([16, D], F32, name=f"out{hb}", tag=f"out{hb}")
                for hb in range(2)
            ]

            # ---- phase A: constants + QKV projection + rope ----
            # sync queue order: wq -> wk -> wv (m-major: the q projection
            # and its rope/pack chain start as soon as wq lands) -> xT ->
            # kt stream
            nc.sync.dma_start(out=xT_sb, in_=xT_d[:])
            wts = [[None, None] for _ in range(3)]
            for m in range(3):
                for h in range(2):
                    wt = wq_pool.tile([128, 8, DPC], BF16, name="wt", tag="wt")
                    nc.sync.dma_start(out=wt, in_=wqkvT_d[:, m, 8 * h : 8 * h + 8, :])
                    wts[m][h] = wt
            nc.scalar.dma_start(out=cos_sb, in_=cosr_d[:])
            nc.scalar.dma_start(out=sin_sb, in_=sinr_d[:])
            nc.scalar.dma_start(out=mask_sb, in_=mask8_d[:])
            # prefetch all of V half-1 up front
            vt = [[None, None] for _ in range(NBP)]
            for bp in range(NBP):
                vt[bp][0] = v_pool.tile([128, 16, 2, DPC], BF16, name="vt", tag="vt")
                nc.scalar.dma_start(out=vt[bp][0], in_=v_d[bp, 0])
            nc.scalar.dma_start(
                out=woT_sb, in_=woT_d.rearrange("(c p) n -> p c n", p=128)
            )
            make_identity(nc, id_sb)

            psA_cm = tc.tile_pool(name="psA", bufs=1, space="PSUM")
            psA = psA_cm.__enter__()
            psT_cm = tc.tile_pool(name="psTA", bufs=2, space="PSUM")
            psT = psT_cm.__enter__()
            ps_q = psA.tile([NTOK, DPC], F32, name="ps_q", tag="ps_q")
            ps_k = psA.tile([NTOK, DPC], F32, name="ps_k", tag="ps_k")
            ps_v = psA.tile([NTOK, DPC], F32, name="ps_v", tag="ps_v")

            cos_r = cos_sb[:].rearrange("p (h i) -> p h i", h=HPC)
            sin_r = sin_sb[:].rearrange("p (h i) -> p h i", h=HPC)
            t0v = rope_t0[:].rearrange("p (h i) -> p h i", h=HPC)
            t1v = rope_t1[:].rearrange("p (h i) -> p h i", h=HPC)

            def proj(m, ps):
                for h in range(2):
                    for i in range(8):
                        c = 8 * h + i
                        nc.tensor.matmul(
                            ps, xT_sb[:, c, :], wts[m][h][:, i, :],
                            start=(c == 0), stop=(c == 15),
                        )

            def rope(ps, dst):
                # rope: projection columns are host-permuted to
                # (head, half, i) so the rotate pairs are contiguous
                # 32-wide blocks
                src = ps[:].rearrange("p (h t i) -> p h t i", h=HPC, t=2)
                dstv = dst[:].rearrange("p (h t i) -> p h t i", h=HPC, t=2)
                t0, t1 = src[:, :, 0, :], src[:, :, 1, :]
                nc.vector.tensor_mul(t0v, t0, cos_r)
                nc.vector.tensor_mul(t1v, t1, sin_r)
                nc.vector.tensor_sub(dstv[:, :, 0, :], t0v, t1v)
                nc.vector.tensor_mul(t0v, t0, sin_r)
                nc.vector.tensor_mul(t1v, t1, cos_r)
                nc.vector.tensor_add(dstv[:, :, 1, :], t0v, t1v)

            def transp(srct, dst):
                for p in range(PAIRS):
                    pt = psT.tile([128, NTOK], F32, name="ptA", tag="ptA")
                    nc.tensor.transpose(
                        pt, srct[:, ts(p, 128)], id_sb[0:NTOK, 0:NTOK]
                    )
                    nc.vector.tensor_copy(dst[p], pt)

            # zero-fill of the QK stationaries depends on nothing - do it
            # first so only the two data copies per (b, p) remain later
            nc.vector.memset(zeros128, 0.0)
            for b in range(B):
                for p in range(PAIRS):
                    nc.vector.tensor_copy(lhsT[b][p], zeros128)

            # q chain first: unblocks the cache QK stream
            proj(0, ps_q)
            rope(ps_q, xq_sb)
            transp(xq_sb, xqT)
            # zero-padded stationary QK operands: lhsT[b][p][dd, col] is
            # nonzero only for col = 16 b + 8 p + 4 h2 + q, h2 = dd // 64
            # (the stationary is zero-padded to all 128 output rows so every
            # (b, p) matmul accumulates into the same full-height PSUM bank)
            for b in range(B):
                for p in range(PAIRS):
                    t = lhsT[b][p]
                    base = 16 * b + 8 * p
                    nc.vector.tensor_copy(
                        t[0:64, base : base + S], xqT[p][0:64, ts(b, S)]
                    )
                    nc.vector.tensor_copy(
                        t[64:128, base + S : base + 8], xqT[p][64:128, ts(b, S)]
                    )
            # k chain
            proj(1, ps_k)
            rope(ps_k, xk_sb)
            transp(xk_sb, xkT)
            # v chain
            proj(2, ps_v)
            nc.vector.tensor_copy(xv32, ps_v)
            for bp in range(NBP):
                # per-bpair value rows relocated to partition base 0 (and
                # interleaved by batch parity on the free axis) so they can
                # be the rhs of the K=4 new-token AV matmul
                for jj in range(2):
                    b = 2 * bp + jj
                    nc.gpsimd.dma_start(
                        out=xvb[bp][:, jj, :], in_=xv32[S * b : S * (b + 1), :]
                    )

            # scores for the 4 new keys: raw scores + mask, exp'd right away
            # (no max subtraction anywhere — see module docstring)
            ps_n = psA.tile([128, S], F32, name="ps_n", tag="ps_n")
            for b in range(B):
                for p in range(PAIRS):
                    nc.tensor.matmul(
                        ps_n,
                        lhsT[b][p][:],
                        xkT[p][:, ts(b, S)],
                        start=(b == 0 and p == 0),
                        stop=(b == B - 1 and p == PAIRS - 1),
                    )
            nc.vector.tensor_add(scores[:, CACHE:KTOT], ps_n, mask_sb)
            nc.scalar.activation(
                scores[:, CACHE:KTOT], scores[:, CACHE:KTOT], EXP,
                scale=0.125, accum_out=rsp[:, NEXP : NEXP + 1],
            )
            ptn = psT.tile([S, 128], F32, name="ptN", tag="ptA")
            nc.tensor.transpose(ptn, scores[:, CACHE:KTOT], id_sb)
            nc.vector.tensor_copy(probsTn, ptn)

            psT_cm.__exit__(None, None, None)
            psA_cm.__exit__(None, None, None)

            # ---- chunked QK -> exp -> transpose -> AV pipeline ----
            psP_cm = tc.tile_pool(name="psP", bufs=1, space="PSUM")
            psP = psP_cm.__enter__()
            pe = [
                psP.tile([32, 2 * DPC], F32, name=f"pe{bp}", tag=f"pe{bp}")
                for bp in range(NBP)
            ]
            psD_cm = tc.tile_pool(name="psD", bufs=2, space="PSUM")
            psD = psD_cm.__enter__()
            psB_cm = tc.tile_pool(name="psB", bufs=1, space="PSUM")
            psB = psB_cm.__enter__()
            psb = [psB.tile([128, 512], F32, name=f"qk{kb}", tag=f"qk{kb}") for kb in range(2)]

            # the new-token AV term opens each accumulation (its probs
            # are ready in phase A), so the final chunk closes with no
            # extra tail matmul
            for bp in range(NBP):
                nc.tensor.matmul(
                    pe[bp],
                    probsTn[:, 32 * bp : 32 * bp + 32],
                    xvb[bp][:],
                    start=True,
                    stop=False,
                )

            def e_chunk(ci, bp, last):
                c0, c1 = CUM[ci], CUM[ci + 1]
                for g in range(c0 // 128, c1 // 128):
                    nc.tensor.matmul(
                        pe[bp],
                        probsT[:, 128 * g + 32 * bp : 128 * g + 32 * bp + 32],
                        vt[bp][g // 16][:, g % 16],
                        start=False,
                        stop=(last and g == c1 // 128 - 1),
                    )

            for ci, csize in enumerate(CHUNKS):
                c0, c1 = CUM[ci], CUM[ci + 1]
                # QK over this chunk: one packed (j, p, k) DMA per batch pair
                for bp in range(NBP):
                    kt = kt_pool.tile([128, 2, 2, 1024], BF16, name="kt", tag="kt")
                    ktv = kt[:, :, :, 0:csize]
                    nc.sync.dma_start(
                        out=ktv, in_=kT_d[bp, :, 4 * c0 : 4 * c1]
                    )
                    for jj in range(2):
                        b = 2 * bp + jj
                        for p in range(PAIRS):
                            for gcol, w, kb, _pi in PIECES[ci]:
                                o = gcol - c0
                                nc.tensor.matmul(
                                    psb[kb][:, 0:w],
                                    lhsT[b][p][:],
                                    ktv[:, jj, p, o : o + w],
                                    start=(bp == 0 and jj == 0 and p == 0),
                                    stop=(bp == NBP - 1 and jj == 1 and p == PAIRS - 1),
                                )
                # exp (scale folded into the activation, no max)
                for gcol, w, kb, pi in PIECES[ci]:
                    nc.scalar.activation(
                        scores[:, gcol : gcol + w], psb[kb][:, 0:w], EXP,
                        scale=0.125, accum_out=rsp[:, pi : pi + 1],
                    )
                if ci == len(CHUNKS) - 1:
                    # final normalization factors; the relocation DMAs run on
                    # gpsimd concurrently with the last AV matmuls
                    nc.vector.reduce_sum(rowsum, rsp[:], axis=AX)
                    nc.vector.reciprocal(recip, rowsum)
                    for b in range(B):
                        bp, jj = b // 2, b % 2
                        nc.gpsimd.dma_start(
                            out=recip_f[16 * jj : 16 * jj + 16, bp : bp + 1],
                            in_=recip[16 * b : 16 * (b + 1), 0:1],
                        )
                # transpose probs chunk -> probsT (copies alternate between
                # DVE and Pool so neither engine becomes the bottleneck)
                for g in range(c0 // 128, c1 // 128):
                    pt = psD.tile([128, 128], F32, name="ptD", tag="ptD")
                    nc.tensor.transpose(pt, scores[:, ts(g, 128)], id_sb)
                    nc.vector.tensor_copy(probsT[:, ts(g, 128)], pt)
                # AV for this chunk (all but the last, which needs psE2/psF)
                if ci < len(CHUNKS) - 1:
                    for bp in range(NBP):
                        e_chunk(ci, bp, last=False)
                        if ci == 1:
                            # second V half arrives while chunk 2 QK streams
                            vt[bp][1] = v_pool.tile(
                                [128, 16, 2, DPC], BF16, name="vt", tag="vt"
                            )
                            nc.scalar.dma_start(out=vt[bp][1], in_=v_d[bp, 1])

            psB_cm.__exit__(None, None, None)
            psD_cm.__exit__(None, None, None)

            # ---- last chunk AV + normalize + attnT + split projection ----
            with (
                tc.tile_pool(name="psE2", bufs=2, space="PSUM") as psE2,
                tc.tile_pool(name="psF", bufs=2, space="PSUM") as psF,
            ):
                def proj_half(hb):
                    # output rows 16*hb .. 16*hb+16 (batches 4*hb..4*hb+4)
                    r0 = 16 * hb
                    for j in range(4):
                        po = psF.tile([16, 512], F32, name="po", tag="po")
                        nc.tensor.matmul(
                            po, attnT_A[:, r0 : r0 + 16], woT_sb[:, 0, ts(j, 512)],
                            start=True, stop=False,
                        )
                        nc.tensor.matmul(
                            po, attnT_B[:, r0 : r0 + 16], woT_sb[:, 1, ts(j, 512)],
                            start=False, stop=True,
                        )
                        nc.vector.tensor_copy(out_sb[hb][:, ts(j, 512)], po)
                    nc.sync.dma_start(
                        out=out_d[r0 : r0 + 16, :], in_=out_sb[hb][:]
                    )

                last_ci = len(CHUNKS) - 1
                # all AV matmuls + scales first, then all transposes: the
                # PE executes strictly in order, so interleaving would
                # bubble it on every DVE scale
                atss = []
                for bp in range(NBP):
                    e_chunk(last_ci, bp, last=True)
                    # engine APs must start at 32-partition boundaries, so
                    # each d-half is scaled as a full 32-row tile (16 rows
                    # are off-quadrant garbage, dropped by the copies below)
                    ats = []
                    for jj in range(2):
                        at = attn_pool.tile(
                            [32, DPC], F32, name="at", tag=f"at{bp}_{jj}"
                        )
                        nc.vector.tensor_scalar_mul(
                            at, in0=pe[bp][:, jj * DPC : (jj + 1) * DPC],
                            scalar1=recip_f[:, bp : bp + 1],
                        )
                        ats.append(at)
                    atss.append(ats)
                for bp in range(NBP):
                    for g in range(2):
                        tgt = attnT_A if g == 0 else attnT_B
                        for jj in range(2):
                            pt32 = psE2.tile([128, 32], F32, name="pt32", tag="pt32")
                            nc.tensor.transpose(
                                pt32, atss[bp][jj][0:32, ts(g, 128)],
                                id_sb[0:32, 0:32],
                            )
                            base = 16 * jj + 8 * g
                            tok = S * (2 * bp + jj)
                            if g == 0:
                                nc.vector.tensor_copy(
                                    tgt[0:64, tok : tok + S],
                                    pt32[0:64, base : base + S],
                                )
                                nc.vector.tensor_copy(
                                    tgt[64:128, tok : tok + S],
                                    pt32[64:128, base + S : base + 8],
                                )
                            else:
                                nc.scalar.mul(
                                    tgt[0:64, tok : tok + S],
                                    pt32[0:64, base : base + S],
                                    1.0,
                                )
                                nc.scalar.mul(
                                    tgt[64:128, tok : tok + S],
                                    pt32[64:128, base + S : base + 8],
                                    1.0,
                                )
                    if bp == 1:
                        proj_half(0)
                    elif bp == 3:
                        proj_half(1)

            psP_cm.__exit__(None, None, None)

    nc.compile()
    _NC_CACHE["nc"] = nc
    return nc


def _rope_perm():
    # projection-output column permutation: (h, d=2i+half) -> (h, half, i)
    perm = np.empty(DPC, np.int64)
    for h in range(HPC):
        for half in range(2):
            for i in range(HD // 2):
                perm[h * HD + half * (HD // 2) + i] = h * HD + 2 * i + half
    return perm


def _prep_in_maps(inputs):
    x = np.ascontiguousarray(np.asarray(inputs["x"], np.float32))
    ck = np.asarray(inputs["cache_k"], np.float32)
    cv = np.asarray(inputs["cache_v"], np.float32)
    wq = np.asarray(inputs["wq"], np.float32)
    wk = np.asarray(inputs["wk"], np.float32)
    wv = np.asarray(inputs["wv"], np.float32)
    wo = np.asarray(inputs["wo"], np.float32)
    fc = np.asarray(inputs["freqs_cos"], np.float32)
    fs = np.asarray(inputs["freqs_sin"], np.float32)
    mask = np.asarray(inputs["mask"], np.float32)

    xT = np.ascontiguousarray(
        x.reshape(NTOK, D).T.reshape(16, 128, NTOK).transpose(1, 0, 2)
    ).astype(NP_BF16)
    cosr = np.ascontiguousarray(np.tile(fc, (B, HPC)))
    sinr = np.ascontiguousarray(np.tile(fs, (B, HPC)))
    mask8n = np.ascontiguousarray(np.tile(mask[0, 0][:, CACHE:] * 8.0, (NTOK, 1)))
    perm = _rope_perm()
    woT = wo.T

    in_maps = []
    for c in range(NCORES):
        hs = slice(HPC * c, HPC * (c + 1))
        ds = slice(DPC * c, DPC * (c + 1))
        wqT = wq[ds].T[:, perm]
        wkT = wk[ds].T[:, perm]
        wvT = wv[ds].T
        # m-major [p, m, c, n]: each m's weights are one contiguous
        # stream so the q chain starts before k/v arrive
        wqkvT = np.ascontiguousarray(
            np.stack(
                [
                    w.reshape(16, 128, DPC).transpose(1, 0, 2)
                    for w in (wqT, wkT, wvT)
                ],
                axis=1,
            )
        ).astype(NP_BF16)
        # [b, k, h, d] head-slice -> [b, pair, (h2, half, i), k]
        cks = ck[:, :, hs, :].reshape(B, CACHE, PAIRS, 2, HD // 2, 2)
        kTf = cks.transpose(0, 2, 3, 5, 4, 1).reshape(B, PAIRS, 128, CACHE)
        # chunk-major, batch-pair packed: [bp, part, (chunk | j, p, k)]
        kq = kTf.reshape(NBP, 2, PAIRS, 128, CACHE)
        blocks = [
            np.ascontiguousarray(
                kq[:, :, :, :, CUM[ci] : CUM[ci + 1]].transpose(0, 3, 1, 2, 4)
            ).reshape(NBP, 128, -1)
            for ci in range(len(CHUNKS))
        ]
        kT = np.ascontiguousarray(np.concatenate(blocks, axis=2)).astype(NP_BF16)
        # [b, hf, chunk, part, d] -> [bpair, hf, part, chunk, j, d]
        v = np.ascontiguousarray(
            cv[:, :, hs, :]
            .reshape(NBP, 2, 2, 16, 128, DPC)
            .transpose(0, 2, 4, 3, 1, 5)
        ).astype(NP_BF16)
        in_maps.append(
            dict(
                xT=xT,
                wqkvT=wqkvT,
                kT=kT,
                v=v,
                mask8n=mask8n,
                cosr=cosr,
                sinr=sinr,
                woT=np.ascontiguousarray(woT[ds]).astype(NP_BF16),
            )
        )
    return in_maps


def run_sharded(inputs, trace=False, **run_kwargs):
    """Build + run on 8 cores; returns (full_output, BassKernelResults)."""
    from concourse.bass_utils import run_bass_kernel_spmd

    nc = _build_nc()
    in_maps = _prep_in_maps(inputs)
    res = run_bass_kernel_spmd(
        nc, in_maps, core_ids=list(range(NCORES)), trace=trace, **run_kwargs
    )
    parts = np.stack([res.results[c]["out"] for c in range(NCORES)])
    out = parts.sum(axis=0, dtype=np.float32).reshape(B, S, D)
    return np.ascontiguousarray(out.astype(np.float32)), res


def kernel(**inputs):
    out, _ = run_sharded(inputs)
    return out
